# revision 9
# baseline (speedup 1.0000x reference)
"""Trainium2 Bass kernel for AFCNet (per-sample 1x1-conv MLP), 8-core data parallel.

Network per sample b (dims 1024 -> 512 -> 256 -> 128 -> 64 -> 1, HW=64):
  q = sigmoid(W1 x + b1); q = q * (drop1 >= .5) * 2
  q = sigmoid(W2 q + b2); q = q * (drop2 >= .5) * 2
  q = sigmoid(W3 q + b3); q = sigmoid(W4 q + b4); out = W5 q + b5

Sharding: batch 64 -> 8 cores x 8 samples (pure data parallel).

v2 design (per-sample software pipeline; v1 did two 4-sample half-waves):
  - All big per-sample blobs (w1x_j, wm_j) ride the SYNC HWDGE ring in
    exact consumption order (one HWDGE queue saturates ~360-400 GB/s;
    v1's scalar-ring-first assumption was inverted by ACT_TABLE_LOADs
    delaying the scalar ring ~2.6us). ScalarE does no DMA arming at all
    so its FIFO is pure ACT work. GpSimd SWDGE carries the small blobs
    (sc, cb, wb).
  - Per-sample waves: iteration j runs L1_j, L2_{j-1}, L3_{j-2}, L4_{j-3}
    on PE with ACT/STT of older samples interleaved on ScalarE/DVE. This
    keeps the PE continuously busy (no >3.4us idle gaps -> HAM stays at
    K=8/8 after warmup; v1 oscillated and ran most MMs at 1.2 GHz) and
    drains the backlog so only the last sample's serial chain trails the
    final DMA byte.
  - One-hot matrices are generated on-chip by DVE memsets (v1 shipped
    them from HBM inside cb). b3 is folded into ACT3's per-partition
    bias operand (L3 output partitions = couts), dropping those bias MMs.
  - L4/L5/out split into two 4-sample chains: chain a completes mid-DMA,
    only chain b is tail.
  - fp8/bf16 numerics identical to v1: fp8e4 weights pre-scaled by pow2
    factors compensated via ACT scale operands; masks {0,2} exact in fp8.
  - PSUM: 8 banks: p1 x2 (j%2), p2 x2 (j%2), p3 x1 (2 slots), p4 x2
    (quads), p5 x1 (both chains + PE warmup group).
"""

import time

import ml_dtypes
import numpy as np

import concourse.tile as tile
from concourse import bacc, mybir
from concourse.bass_utils import run_bass_kernel_spmd

N_CORES = 8
S = 8            # samples per core
HW = 64
F8NP = ml_dtypes.float8_e4m3
BFNP = ml_dtypes.bfloat16

BF16 = mybir.dt.bfloat16
F8 = mybir.dt.float8e4
F32 = mybir.dt.float32
SIG = mybir.ActivationFunctionType.Sigmoid
MULT = mybir.AluOpType.mult

# --- w1x blob columns (fp8): w1T chunks (k0..7, m0..3)*128, then x chunks ---
X_OFF = 4096                    # 8 chunks x 64
W1X_COLS = 4608
# --- wm blob columns (fp8): w2T (k0..3, m0..1)*128, w3T (k0..1)*128, masks ---
W3_OFF = 1024
MK1_OFF = 1280                  # mask1 [128, 4*64]
MK2_OFF = 1536                  # mask2 [128, 2*64]
WM_COLS = 1664
# --- wb blob (bf16) [128, S*65]: per sample w4T(64) + w5col(1) ---
WB_PER = 65
WB_COLS = S * WB_PER
# --- cb blob (bf16) [8, 2690]: one-hot + per-sample bias lhsT tiles ---
OH_OFF = 0                      # rows 0-7: block-diagonal one-hot [8, 512]
B1_OFF = 512                    # rows 0-3: b1_j [4,128] at cols j*128
B2_OFF = 1536                   # rows 0-1: b2_j [2,128]
B4_OFF = 2560                   # rows 0-3: b4 quad a [4,64], quad b [4,64]
B5_OFF = 2688                   # rows 0-3: b5 quad a [4,1], quad b [4,1]
CB_COLS = 2690
# --- sc blob (f32) [128, 11]: 3 act scales + 8 per-sample b3 columns ---
SC_COLS = 11

_COMPILED = None
LAST_RESULT = None


def _build():
    nc = bacc.Bacc(target_bir_lowering=False)
    w1x_d = nc.declare_dram_parameter("w1x", [S, 128, W1X_COLS], F8, isOutput=False)
    wm_d = nc.declare_dram_parameter("wm", [S, 128, WM_COLS], F8, isOutput=False)
    cb_d = nc.declare_dram_parameter("cb", [8, CB_COLS], BF16, isOutput=False)
    sc_d = nc.declare_dram_parameter("sc", [128, SC_COLS], F32, isOutput=False)
    wb_d = nc.declare_dram_parameter("wb", [128, WB_COLS], BF16, isOutput=False)
    out_d = nc.declare_dram_parameter("out", [1, S * HW], F32, isOutput=True)

    with tile.TileContext(nc) as tc:
        with (
            tc.tile_pool(name="sbuf", bufs=1) as sb,
            tc.tile_pool(name="psum", bufs=1, space="PSUM") as ps,
        ):
            scT = sb.tile([128, SC_COLS], F32, tag="scT")
            cbT = sb.tile([8, CB_COLS], BF16, tag="cbT")
            wbT = sb.tile([128, WB_COLS], BF16, tag="wbT")
            warm = sb.tile([128, HW], BF16, tag="warm")

            w1xT = [sb.tile([128, W1X_COLS], F8, tag=f"w1x{j}", name=f"w1x{j}")
                    for j in range(S)]
            wmT = [sb.tile([128, WM_COLS], F8, tag=f"wm{j}", name=f"wm{j}")
                   for j in range(S)]
            q1 = [sb.tile([128, 4, HW], BF16, tag=f"q1_{j}", name=f"q1_{j}")
                  for j in range(S)]
            q2 = [sb.tile([128, 2, HW], BF16, tag=f"q2_{j}", name=f"q2_{j}")
                  for j in range(S)]
            q3 = [sb.tile([128, HW], BF16, tag=f"q3_{j}", name=f"q3_{j}")
                  for j in range(S)]
            q4 = [sb.tile([64, 4, HW], BF16, tag=f"q4_{h}", name=f"q4_{h}")
                  for h in range(2)]
            outs = [sb.tile([1, 4, HW], F32, tag=f"out_{h}", name=f"out_{h}")
                    for h in range(2)]

            # small blobs on gpsimd SWDGE; consumed from ~10us on.
            nc.gpsimd.dma_start(out=scT[:], in_=sc_d[:, :])
            nc.gpsimd.dma_start(out=cbT[:], in_=cb_d[:, :])
            nc.gpsimd.dma_start(out=wbT[:], in_=wb_d[:, :])

            # DVE preamble: warmup operand.
            nc.vector.memset(warm[:], 0.0)
            ohT = cbT[0:8, OH_OFF:OH_OFF + 512]

            # All big blobs on the sync HWDGE ring in consumption order.
            for j in range(S):
                nc.sync.dma_start(out=w1xT[j][:], in_=w1x_d[j, :, :])
                nc.sync.dma_start(out=wmT[j][:], in_=wm_d[j, :, :])

            # PSUM: 8 banks (padded to a full bank each).
            p1 = [ps.tile([128, 4, HW], F32, tag=f"p1_{s}", name=f"p1_{s}",
                          padded_shape=[128, 4, 2 * HW]) for s in range(2)]
            p2 = [ps.tile([128, 2, HW], F32, tag=f"p2_{s}", name=f"p2_{s}",
                          padded_shape=[128, 2, 4 * HW]) for s in range(2)]
            p3 = ps.tile([128, 2, HW], F32, tag="p3",
                         padded_shape=[128, 2, 4 * HW])
            p4 = [ps.tile([64, 4, HW], F32, tag=f"p4_{h}", name=f"p4_{h}",
                          padded_shape=[64, 4, 2 * HW]) for h in range(2)]
            p5 = ps.tile([1, 2, 4, HW], F32, tag="p5")

            # PE warmup: one long accumulation group keeps HAM from
            # throttling while the first weight DMA is in flight.
            for i in range(64):
                nc.tensor.matmul(p5[:, 0, 0, :], warm[:, 0:1], warm[:],
                                 start=(i == 0), stop=(i == 63),
                                 skip_group_check=True)

            sc1 = scT[:, 0:1]
            sc2 = scT[:, 1:2]
            sc3 = scT[:, 2:3]

            def l1(j):
                pt = p1[j % 2]
                wt = w1xT[j]
                for m in range(4):
                    for k in range(8):
                        nc.tensor.matmul(
                            pt[:, m, :],
                            wt[:, (k * 4 + m) * 128:(k * 4 + m + 1) * 128],
                            wt[:, X_OFF + k * HW:X_OFF + (k + 1) * HW],
                            start=(m == 0 and k == 0), stop=False,
                            skip_group_check=True)
                nc.tensor.matmul(
                    pt[:, :, :],
                    cbT[0:4, B1_OFF + j * 128:B1_OFF + (j + 1) * 128],
                    ohT[0:4, 0:256],
                    start=False, stop=True, skip_group_check=True)

            def act1(j):
                nc.scalar.activation(q1[j][:, :, :], p1[j % 2][:, :, :],
                                     SIG, scale=sc1)
                nc.vector.scalar_tensor_tensor(
                    out=q1[j][:, :, :],
                    in0=wmT[j][:, MK1_OFF:MK1_OFF + 256].rearrange(
                        "p (m t) -> p m t", m=4),
                    scalar=1.0, in1=q1[j][:, :, :], op0=MULT, op1=MULT)

            def l2(j):
                pt = p2[j % 2]
                wt = wmT[j]
                for m in range(2):
                    for k in range(4):
                        nc.tensor.matmul(
                            pt[:, m, :],
                            wt[:, (k * 2 + m) * 128:(k * 2 + m + 1) * 128],
                            q1[j][:, k, :],
                            start=(m == 0 and k == 0), stop=False,
                            skip_group_check=True)
                nc.tensor.matmul(
                    pt[:, :, :],
                    cbT[0:2, B2_OFF + j * 128:B2_OFF + (j + 1) * 128],
                    ohT[0:2, 0:128],
                    start=False, stop=True, skip_group_check=True)

            def act2(j):
                nc.scalar.activation(q2[j][:, :, :], p2[j % 2][:, :, :],
                                     SIG, scale=sc2)
                nc.vector.scalar_tensor_tensor(
                    out=q2[j][:, :, :],
                    in0=wmT[j][:, MK2_OFF:MK2_OFF + 128].rearrange(
                        "p (m t) -> p m t", m=2),
                    scalar=1.0, in1=q2[j][:, :, :], op0=MULT, op1=MULT)

            def l3(j):
                for k in range(2):
                    nc.tensor.matmul(
                        p3[:, j % 2, :],
                        wmT[j][:, W3_OFF + k * 128:W3_OFF + (k + 1) * 128],
                        q2[j][:, k, :],
                        start=(k == 0), stop=(k == 1), skip_group_check=True)

            def act3(j):
                # b3 folded into the per-partition bias operand.
                nc.scalar.activation(q3[j][:, :], p3[:, j % 2, :], SIG,
                                     bias=scT[:, 3 + j:4 + j], scale=sc3)

            def l4(j):
                h, i = j // 4, j % 4
                nc.tensor.matmul(
                    p4[h][:, i, :], wbT[:, j * WB_PER:j * WB_PER + 64],
                    q3[j][:, :], start=(i == 0), stop=False,
                    skip_group_check=True)
                if i == 3:
                    nc.tensor.matmul(
                        p4[h][:, :, :],
                        cbT[0:4, B4_OFF + h * 64:B4_OFF + (h + 1) * 64],
                        ohT[0:4, 0:256],
                        start=False, stop=True, skip_group_check=True)
                    nc.scalar.activation(q4[h][:, :, :], p4[h][:, :, :], SIG)

            def l5(h):
                for i in range(4):
                    j = 4 * h + i
                    nc.tensor.matmul(
                        p5[:, h, i, :],
                        wbT[0:64, j * WB_PER + 64:j * WB_PER + 65],
                        q4[h][:, i, :], start=(i == 0), stop=False,
                        skip_group_check=True)
                nc.tensor.matmul(
                    p5[:, h, :, :], cbT[0:4, B5_OFF + h:B5_OFF + h + 1],
                    ohT[0:4, 0:256],
                    start=False, stop=True, skip_group_check=True)
                nc.vector.tensor_scalar_mul(outs[h][:, :, :],
                                            p5[:, h, :, :], 1.0)
                nc.sync.dma_start(
                    out=out_d[0:1, h * 256:(h + 1) * 256],
                    in_=outs[h].rearrange("p a b -> p (a b)"))

            for it in range(11):
                if it < 8:
                    l1(it)
                if 0 <= it - 1 < 8:
                    act1(it - 1)
                    l2(it - 1)
                if 0 <= it - 2 < 8:
                    act2(it - 2)
                    l3(it - 2)
                if 0 <= it - 3 < 8:
                    act3(it - 3)
                    l4(it - 3)
                if it - 3 == 4:
                    l5(0)       # chain a completes mid-DMA
            l5(1)
    nc.compile()
    return nc


def _pow2_scale(a, cap=224.0):
    m = float(np.abs(a).max())
    if m == 0.0:
        return 1.0
    return float(2.0 ** np.floor(np.log2(cap / m)))


def _pack(x, w1, b1, w2, b2, w3, b3, w4, b4, w5, b5, drop1, drop2):
    """Build per-sample w1x/wm blobs; return bias/scale data."""
    B = x.shape[0]
    f4 = np.float32
    x3 = np.ascontiguousarray(x.reshape(B, 1024, HW), dtype=f4)
    w1m = w1.reshape(B, 512, 1024).astype(f4, copy=False)
    w2m = w2.reshape(B, 256, 512).astype(f4, copy=False)
    w3m = w3.reshape(B, 128, 256).astype(f4, copy=False)
    w4m = w4.reshape(B, 64, 128).astype(f4, copy=False)
    w5m = w5.reshape(B, 64).astype(f4, copy=False)

    sx = _pow2_scale(x3)
    s1 = _pow2_scale(w1m)
    s2 = _pow2_scale(w2m)
    s3 = _pow2_scale(w3m)

    def chunkT(wT, nk, nm):  # [B, cin, cout] -> [B, 128, nk*nm*128]
        Bn, cin, cout = wT.shape
        return np.ascontiguousarray(
            wT.reshape(Bn, nk, 128, nm, 128).transpose(0, 2, 1, 3, 4)
        ).reshape(Bn, 128, nk * nm * 128)

    w1T = chunkT(np.swapaxes(w1m, 1, 2) * s1, 8, 4)
    xc = np.ascontiguousarray(
        x3.reshape(B, 8, 128, HW).transpose(0, 2, 1, 3)).reshape(B, 128, 512) * sx
    w1x = np.concatenate([w1T, xc], axis=2).astype(F8NP)

    w2T = chunkT(np.swapaxes(w2m, 1, 2) * s2, 4, 2)
    w3T = chunkT(np.swapaxes(w3m, 1, 2) * s3, 2, 1)
    m1 = (drop1.reshape(B, 512, HW) >= np.float32(0.5)).astype(f4) * f4(2.0)
    m1 = np.ascontiguousarray(
        m1.reshape(B, 4, 128, HW).transpose(0, 2, 1, 3)).reshape(B, 128, 256)
    m2 = (drop2.reshape(B, 256, HW) >= np.float32(0.5)).astype(f4) * f4(2.0)
    m2 = np.ascontiguousarray(
        m2.reshape(B, 2, 128, HW).transpose(0, 2, 1, 3)).reshape(B, 128, 128)
    wm = np.concatenate([w2T, w3T, m1, m2], axis=2).astype(F8NP)

    wb = np.zeros((B, 128, WB_PER), f4)
    wb[:, :, 0:64] = np.swapaxes(w4m, 1, 2)
    wb[:, :64, 64] = w5m

    b1s = b1.astype(f4) * f4(s1 * sx)
    b2s = b2.astype(f4) * f4(s2)
    scales = (1.0 / (s1 * sx), 1.0 / s2, 1.0 / s3)
    return w1x, wm, wb, (b1s, b2s, b3.astype(f4), b4.astype(f4),
                         b5.reshape(B).astype(f4)), scales


def kernel(**inputs):
    global _COMPILED, LAST_RESULT
    if _COMPILED is None:
        _COMPILED = _build()
    nc = _COMPILED

    w1x, wm, wb, (b1s, b2s, b3f, b4f, b5f), scales = _pack(
        **{k: np.asarray(v) for k, v in inputs.items()})

    in_maps = []
    for c in range(N_CORES):
        sl = slice(c * S, (c + 1) * S)
        wbc = wb[sl].transpose(1, 0, 2).reshape(128, S * WB_PER)

        cb = np.zeros((8, CB_COLS), np.float32)
        for r in range(8):
            cb[r, OH_OFF + r * HW:OH_OFF + (r + 1) * HW] = 1.0
        for j in range(S):
            g = c * S + j
            cb[0:4, B1_OFF + j * 128:B1_OFF + (j + 1) * 128] = \
                b1s[g].reshape(4, 128)
            cb[0:2, B2_OFF + j * 128:B2_OFF + (j + 1) * 128] = \
                b2s[g].reshape(2, 128)
        cb[0:4, B4_OFF:B4_OFF + 64] = b4f[c * S:c * S + 4]
        cb[0:4, B4_OFF + 64:B4_OFF + 128] = b4f[c * S + 4:c * S + 8]
        cb[0:4, B5_OFF] = b5f[c * S:c * S + 4]
        cb[0:4, B5_OFF + 1] = b5f[c * S + 4:c * S + 8]

        scc = np.empty((128, SC_COLS), np.float32)
        scc[:, 0] = scales[0]
        scc[:, 1] = scales[1]
        scc[:, 2] = scales[2]
        for j in range(S):
            scc[:, 3 + j] = b3f[c * S + j]

        in_maps.append({
            "w1x": np.ascontiguousarray(w1x[sl]),
            "wm": np.ascontiguousarray(wm[sl]),
            "cb": cb.astype(BFNP),
            "sc": scc,
            "wb": np.ascontiguousarray(wbc).astype(BFNP),
        })

    res = None
    for attempt in range(3):
        try:
            res = run_bass_kernel_spmd(nc, in_maps, core_ids=list(range(N_CORES)))
            break
        except Exception:
            if attempt == 2:
                raise
            time.sleep(20)
            try:  # best-effort device reconnect after NRT_EXEC_UNIT_UNRECOVERABLE
                import jax
                jax.clear_caches()
                import jax.extend.backend as _jeb
                _jeb.clear_backends()
            except Exception:
                pass
    LAST_RESULT = res
    outs = [np.asarray(res.results[c]["out"]).reshape(S, 8, 8)
            for c in range(N_CORES)]
    return np.concatenate(outs, axis=0).astype(np.float32)


# revision 11
# speedup vs baseline: 1.0030x; 1.0030x over previous
"""Trainium2 Bass kernel for AFCNet (per-sample 1x1-conv MLP), 8-core data parallel.

Network per sample b (dims 1024 -> 512 -> 256 -> 128 -> 64 -> 1, HW=64):
  q = sigmoid(W1 x + b1); q = q * (drop1 >= .5) * 2
  q = sigmoid(W2 q + b2); q = q * (drop2 >= .5) * 2
  q = sigmoid(W3 q + b3); q = sigmoid(W4 q + b4); out = W5 q + b5

Sharding: batch 64 -> 8 cores x 8 samples (pure data parallel).

v2 design (per-sample software pipeline; v1 did two 4-sample half-waves):
  - All big per-sample blobs (w1x_j, wm_j) ride the SYNC HWDGE ring in
    exact consumption order (one HWDGE queue saturates ~360-400 GB/s;
    v1's scalar-ring-first assumption was inverted by ACT_TABLE_LOADs
    delaying the scalar ring ~2.6us). ScalarE does no DMA arming at all
    so its FIFO is pure ACT work. GpSimd SWDGE carries the small blobs
    (sc, cb, wb).
  - Per-sample waves: iteration j runs L1_j, L2_{j-1}, L3_{j-2}, L4_{j-3}
    on PE with ACT/STT of older samples interleaved on ScalarE/DVE. This
    keeps the PE continuously busy (no >3.4us idle gaps -> HAM stays at
    K=8/8 after warmup; v1 oscillated and ran most MMs at 1.2 GHz) and
    drains the backlog so only the last sample's serial chain trails the
    final DMA byte.
  - One-hot matrices are generated on-chip by DVE memsets (v1 shipped
    them from HBM inside cb). b3 is folded into ACT3's per-partition
    bias operand (L3 output partitions = couts), dropping those bias MMs.
  - L4/L5/out split into two 4-sample chains: chain a completes mid-DMA,
    only chain b is tail.
  - fp8/bf16 numerics identical to v1: fp8e4 weights pre-scaled by pow2
    factors compensated via ACT scale operands; masks {0,2} exact in fp8.
  - PSUM: 8 banks: p1 x2 (j%2), p2 x2 (j%2), p3 x1 (2 slots), p4 x2
    (quads), p5 x1 (both chains + PE warmup group).
"""

import time

import ml_dtypes
import numpy as np

import concourse.tile as tile
from concourse import bacc, mybir
from concourse.bass_utils import run_bass_kernel_spmd

N_CORES = 8
S = 8            # samples per core
HW = 64
F8NP = ml_dtypes.float8_e4m3
BFNP = ml_dtypes.bfloat16

BF16 = mybir.dt.bfloat16
F8 = mybir.dt.float8e4
F32 = mybir.dt.float32
SIG = mybir.ActivationFunctionType.Sigmoid
MULT = mybir.AluOpType.mult

# --- w1x blob columns (fp8): w1T chunks (k0..7, m0..3)*128, then x chunks ---
X_OFF = 4096                    # 8 chunks x 64
W1X_COLS = 4608
# --- wm blob columns (fp8): w2T (k0..3, m0..1)*128, w3T (k0..1)*128, masks ---
W3_OFF = 1024
MK1_OFF = 1280                  # mask1 [128, 4*64]
MK2_OFF = 1536                  # mask2 [128, 2*64]
WM_COLS = 1664
# --- wb blob (bf16) [128, S*65]: per sample w4T(64) + w5col(1) ---
WB_PER = 65
WB_COLS = S * WB_PER
# --- cb blob (bf16) [8, 2690]: one-hot + per-sample bias lhsT tiles ---
OH_OFF = 0                      # rows 0-7: block-diagonal one-hot [8, 512]
B1_OFF = 512                    # rows 0-3: b1_j [4,128] at cols j*128
B2_OFF = 1536                   # rows 0-1: b2_j [2,128]
B4_OFF = 2560                   # rows 0-3: b4 quad a [4,64], quad b [4,64]
B5_OFF = 2688                   # rows 0-3: b5 quad a [4,1], quad b [4,1]
CB_COLS = 2690
# --- sc blob (f32) [128, 11]: 3 act scales + 8 per-sample b3 columns ---
SC_COLS = 11

_COMPILED = None
LAST_RESULT = None


def _build():
    nc = bacc.Bacc(target_bir_lowering=False)
    w1x_d = nc.declare_dram_parameter("w1x", [S, 128, W1X_COLS], F8, isOutput=False)
    wm_d = nc.declare_dram_parameter("wm", [S, 128, WM_COLS], F8, isOutput=False)
    cb_d = nc.declare_dram_parameter("cb", [8, CB_COLS], BF16, isOutput=False)
    sc_d = nc.declare_dram_parameter("sc", [128, SC_COLS], F32, isOutput=False)
    wb_d = nc.declare_dram_parameter("wb", [128, WB_COLS], BF16, isOutput=False)
    out_d = nc.declare_dram_parameter("out", [1, S * HW], F32, isOutput=True)

    with tile.TileContext(nc) as tc:
        with (
            tc.tile_pool(name="sbuf", bufs=1) as sb,
            tc.tile_pool(name="psum", bufs=1, space="PSUM") as ps,
        ):
            scT = sb.tile([128, SC_COLS], F32, tag="scT")
            cbT = sb.tile([8, CB_COLS], BF16, tag="cbT")
            wbT = sb.tile([128, WB_COLS], BF16, tag="wbT")
            warm = sb.tile([128, HW], BF16, tag="warm")

            w1xT = [sb.tile([128, W1X_COLS], F8, tag=f"w1x{j}", name=f"w1x{j}")
                    for j in range(S)]
            wmT = [sb.tile([128, WM_COLS], F8, tag=f"wm{j}", name=f"wm{j}")
                   for j in range(S)]
            q1 = [sb.tile([128, 4, HW], BF16, tag=f"q1_{j}", name=f"q1_{j}")
                  for j in range(S)]
            q2 = [sb.tile([128, 2, HW], BF16, tag=f"q2_{j}", name=f"q2_{j}")
                  for j in range(S)]
            q3 = [sb.tile([128, HW], BF16, tag=f"q3_{j}", name=f"q3_{j}")
                  for j in range(S)]
            q4 = [sb.tile([64, 4, HW], BF16, tag=f"q4_{h}", name=f"q4_{h}")
                  for h in range(2)]
            outs = [sb.tile([1, 4, HW], F32, tag=f"out_{h}", name=f"out_{h}")
                    for h in range(2)]

            # Small blobs ride the scalar HWDGE ring (otherwise idle for
            # DMA): gpsimd SWDGE would contend for the same SDMA engines
            # as the sync ring and straggle the w1x0 completion sem by
            # ~2us (cb is an 8-partition blob -> 1-2 engines only).
            nc.scalar.dma_start(out=cbT[:], in_=cb_d[:, :])
            nc.scalar.dma_start(out=scT[:], in_=sc_d[:, :])
            nc.scalar.dma_start(out=wbT[:], in_=wb_d[:, :])

            # DVE preamble: warmup operand.
            nc.vector.memset(warm[:], 0.0)
            ohT = cbT[0:8, OH_OFF:OH_OFF + 512]

            # All big blobs on the sync HWDGE ring in consumption order.
            for j in range(S):
                nc.sync.dma_start(out=w1xT[j][:], in_=w1x_d[j, :, :])
                nc.sync.dma_start(out=wmT[j][:], in_=wm_d[j, :, :])

            # PSUM: 8 banks (padded to a full bank each).
            p1 = [ps.tile([128, 4, HW], F32, tag=f"p1_{s}", name=f"p1_{s}",
                          padded_shape=[128, 4, 2 * HW]) for s in range(2)]
            p2 = [ps.tile([128, 2, HW], F32, tag=f"p2_{s}", name=f"p2_{s}",
                          padded_shape=[128, 2, 4 * HW]) for s in range(2)]
            p3 = ps.tile([128, 2, HW], F32, tag="p3",
                         padded_shape=[128, 2, 4 * HW])
            p4 = [ps.tile([64, 4, HW], F32, tag=f"p4_{h}", name=f"p4_{h}",
                          padded_shape=[64, 4, 2 * HW]) for h in range(2)]
            p5 = ps.tile([1, 2, 4, HW], F32, tag="p5")

            # PE warmup: one long accumulation group keeps HAM from
            # throttling while the first weight DMA is in flight.
            for i in range(64):
                nc.tensor.matmul(p5[:, 0, 0, :], warm[:, 0:1], warm[:],
                                 start=(i == 0), stop=(i == 63),
                                 skip_group_check=True)

            sc1 = scT[:, 0:1]
            sc2 = scT[:, 1:2]
            sc3 = scT[:, 2:3]

            def l1(j):
                pt = p1[j % 2]
                wt = w1xT[j]
                for m in range(4):
                    for k in range(8):
                        nc.tensor.matmul(
                            pt[:, m, :],
                            wt[:, (k * 4 + m) * 128:(k * 4 + m + 1) * 128],
                            wt[:, X_OFF + k * HW:X_OFF + (k + 1) * HW],
                            start=(m == 0 and k == 0), stop=False,
                            skip_group_check=True)
                nc.tensor.matmul(
                    pt[:, :, :],
                    cbT[0:4, B1_OFF + j * 128:B1_OFF + (j + 1) * 128],
                    ohT[0:4, 0:256],
                    start=False, stop=True, skip_group_check=True)

            def act1(j):
                nc.scalar.activation(q1[j][:, :, :], p1[j % 2][:, :, :],
                                     SIG, scale=sc1)
                nc.vector.scalar_tensor_tensor(
                    out=q1[j][:, :, :],
                    in0=wmT[j][:, MK1_OFF:MK1_OFF + 256].rearrange(
                        "p (m t) -> p m t", m=4),
                    scalar=1.0, in1=q1[j][:, :, :], op0=MULT, op1=MULT)

            def l2(j):
                pt = p2[j % 2]
                wt = wmT[j]
                for m in range(2):
                    for k in range(4):
                        nc.tensor.matmul(
                            pt[:, m, :],
                            wt[:, (k * 2 + m) * 128:(k * 2 + m + 1) * 128],
                            q1[j][:, k, :],
                            start=(m == 0 and k == 0), stop=False,
                            skip_group_check=True)
                nc.tensor.matmul(
                    pt[:, :, :],
                    cbT[0:2, B2_OFF + j * 128:B2_OFF + (j + 1) * 128],
                    ohT[0:2, 0:128],
                    start=False, stop=True, skip_group_check=True)

            def act2(j):
                nc.scalar.activation(q2[j][:, :, :], p2[j % 2][:, :, :],
                                     SIG, scale=sc2)
                nc.vector.scalar_tensor_tensor(
                    out=q2[j][:, :, :],
                    in0=wmT[j][:, MK2_OFF:MK2_OFF + 128].rearrange(
                        "p (m t) -> p m t", m=2),
                    scalar=1.0, in1=q2[j][:, :, :], op0=MULT, op1=MULT)

            def l3(j):
                for k in range(2):
                    nc.tensor.matmul(
                        p3[:, j % 2, :],
                        wmT[j][:, W3_OFF + k * 128:W3_OFF + (k + 1) * 128],
                        q2[j][:, k, :],
                        start=(k == 0), stop=(k == 1), skip_group_check=True)

            def act3(j):
                # b3 folded into the per-partition bias operand.
                nc.scalar.activation(q3[j][:, :], p3[:, j % 2, :], SIG,
                                     bias=scT[:, 3 + j:4 + j], scale=sc3)

            def l4(j):
                h, i = j // 4, j % 4
                nc.tensor.matmul(
                    p4[h][:, i, :], wbT[:, j * WB_PER:j * WB_PER + 64],
                    q3[j][:, :], start=(i == 0), stop=False,
                    skip_group_check=True)
                if i == 3:
                    nc.tensor.matmul(
                        p4[h][:, :, :],
                        cbT[0:4, B4_OFF + h * 64:B4_OFF + (h + 1) * 64],
                        ohT[0:4, 0:256],
                        start=False, stop=True, skip_group_check=True)
                    nc.scalar.activation(q4[h][:, :, :], p4[h][:, :, :], SIG)

            def l5(h):
                for i in range(4):
                    j = 4 * h + i
                    nc.tensor.matmul(
                        p5[:, h, i, :],
                        wbT[0:64, j * WB_PER + 64:j * WB_PER + 65],
                        q4[h][:, i, :], start=(i == 0), stop=False,
                        skip_group_check=True)
                nc.tensor.matmul(
                    p5[:, h, :, :], cbT[0:4, B5_OFF + h:B5_OFF + h + 1],
                    ohT[0:4, 0:256],
                    start=False, stop=True, skip_group_check=True)
                nc.vector.tensor_scalar_mul(outs[h][:, :, :],
                                            p5[:, h, :, :], 1.0)
                nc.sync.dma_start(
                    out=out_d[0:1, h * 256:(h + 1) * 256],
                    in_=outs[h].rearrange("p a b -> p (a b)"))

            # Deep software pipeline: every cross-engine dependency is at
            # least one full iteration (~2.2us) old by the time its PE
            # consumer can issue, so scheduler reorderings can't stall
            # the PE FIFO (v2 used 1-iteration offsets and lost ~730ns
            # per iteration to ACT/STT head-of-line waits).
            for it in range(12):
                if it < 8:
                    l1(it)
                if 0 <= it - 1 < 8:
                    act1(it - 1)
                if 0 <= it - 2 < 8:
                    l2(it - 2)
                    act2(it - 2)
                if 0 <= it - 3 < 8:
                    l3(it - 3)
                    act3(it - 3)
                if 0 <= it - 4 < 8:
                    l4(it - 4)
                if it - 4 == 3:
                    l5(0)       # chain a completes mid-DMA
            l5(1)
    nc.compile()
    return nc


def _pow2_scale(a, cap=224.0):
    m = float(np.abs(a).max())
    if m == 0.0:
        return 1.0
    return float(2.0 ** np.floor(np.log2(cap / m)))


def _pack(x, w1, b1, w2, b2, w3, b3, w4, b4, w5, b5, drop1, drop2):
    """Build per-sample w1x/wm blobs; return bias/scale data."""
    B = x.shape[0]
    f4 = np.float32
    x3 = np.ascontiguousarray(x.reshape(B, 1024, HW), dtype=f4)
    w1m = w1.reshape(B, 512, 1024).astype(f4, copy=False)
    w2m = w2.reshape(B, 256, 512).astype(f4, copy=False)
    w3m = w3.reshape(B, 128, 256).astype(f4, copy=False)
    w4m = w4.reshape(B, 64, 128).astype(f4, copy=False)
    w5m = w5.reshape(B, 64).astype(f4, copy=False)

    sx = _pow2_scale(x3)
    s1 = _pow2_scale(w1m)
    s2 = _pow2_scale(w2m)
    s3 = _pow2_scale(w3m)

    def chunkT(wT, nk, nm):  # [B, cin, cout] -> [B, 128, nk*nm*128]
        Bn, cin, cout = wT.shape
        return np.ascontiguousarray(
            wT.reshape(Bn, nk, 128, nm, 128).transpose(0, 2, 1, 3, 4)
        ).reshape(Bn, 128, nk * nm * 128)

    w1T = chunkT(np.swapaxes(w1m, 1, 2) * s1, 8, 4)
    xc = np.ascontiguousarray(
        x3.reshape(B, 8, 128, HW).transpose(0, 2, 1, 3)).reshape(B, 128, 512) * sx
    w1x = np.concatenate([w1T, xc], axis=2).astype(F8NP)

    w2T = chunkT(np.swapaxes(w2m, 1, 2) * s2, 4, 2)
    w3T = chunkT(np.swapaxes(w3m, 1, 2) * s3, 2, 1)
    m1 = (drop1.reshape(B, 512, HW) >= np.float32(0.5)).astype(f4) * f4(2.0)
    m1 = np.ascontiguousarray(
        m1.reshape(B, 4, 128, HW).transpose(0, 2, 1, 3)).reshape(B, 128, 256)
    m2 = (drop2.reshape(B, 256, HW) >= np.float32(0.5)).astype(f4) * f4(2.0)
    m2 = np.ascontiguousarray(
        m2.reshape(B, 2, 128, HW).transpose(0, 2, 1, 3)).reshape(B, 128, 128)
    wm = np.concatenate([w2T, w3T, m1, m2], axis=2).astype(F8NP)

    wb = np.zeros((B, 128, WB_PER), f4)
    wb[:, :, 0:64] = np.swapaxes(w4m, 1, 2)
    wb[:, :64, 64] = w5m

    b1s = b1.astype(f4) * f4(s1 * sx)
    b2s = b2.astype(f4) * f4(s2)
    scales = (1.0 / (s1 * sx), 1.0 / s2, 1.0 / s3)
    return w1x, wm, wb, (b1s, b2s, b3.astype(f4), b4.astype(f4),
                         b5.reshape(B).astype(f4)), scales


def kernel(**inputs):
    global _COMPILED, LAST_RESULT
    if _COMPILED is None:
        _COMPILED = _build()
    nc = _COMPILED

    w1x, wm, wb, (b1s, b2s, b3f, b4f, b5f), scales = _pack(
        **{k: np.asarray(v) for k, v in inputs.items()})

    in_maps = []
    for c in range(N_CORES):
        sl = slice(c * S, (c + 1) * S)
        wbc = wb[sl].transpose(1, 0, 2).reshape(128, S * WB_PER)

        cb = np.zeros((8, CB_COLS), np.float32)
        for r in range(8):
            cb[r, OH_OFF + r * HW:OH_OFF + (r + 1) * HW] = 1.0
        for j in range(S):
            g = c * S + j
            cb[0:4, B1_OFF + j * 128:B1_OFF + (j + 1) * 128] = \
                b1s[g].reshape(4, 128)
            cb[0:2, B2_OFF + j * 128:B2_OFF + (j + 1) * 128] = \
                b2s[g].reshape(2, 128)
        cb[0:4, B4_OFF:B4_OFF + 64] = b4f[c * S:c * S + 4]
        cb[0:4, B4_OFF + 64:B4_OFF + 128] = b4f[c * S + 4:c * S + 8]
        cb[0:4, B5_OFF] = b5f[c * S:c * S + 4]
        cb[0:4, B5_OFF + 1] = b5f[c * S + 4:c * S + 8]

        scc = np.empty((128, SC_COLS), np.float32)
        scc[:, 0] = scales[0]
        scc[:, 1] = scales[1]
        scc[:, 2] = scales[2]
        for j in range(S):
            scc[:, 3 + j] = b3f[c * S + j]

        in_maps.append({
            "w1x": np.ascontiguousarray(w1x[sl]),
            "wm": np.ascontiguousarray(wm[sl]),
            "cb": cb.astype(BFNP),
            "sc": scc,
            "wb": np.ascontiguousarray(wbc).astype(BFNP),
        })

    res = None
    for attempt in range(3):
        try:
            res = run_bass_kernel_spmd(nc, in_maps, core_ids=list(range(N_CORES)))
            break
        except Exception:
            if attempt == 2:
                raise
            time.sleep(20)
            try:  # best-effort device reconnect after NRT_EXEC_UNIT_UNRECOVERABLE
                import jax
                jax.clear_caches()
                import jax.extend.backend as _jeb
                _jeb.clear_backends()
            except Exception:
                pass
    LAST_RESULT = res
    outs = [np.asarray(res.results[c]["out"]).reshape(S, 8, 8)
            for c in range(N_CORES)]
    return np.concatenate(outs, axis=0).astype(np.float32)


# revision 13
# speedup vs baseline: 1.0628x; 1.0596x over previous
"""Trainium2 Bass kernel for AFCNet (per-sample 1x1-conv MLP), 8-core data parallel.

Network per sample b (dims 1024 -> 512 -> 256 -> 128 -> 64 -> 1, HW=64):
  q = sigmoid(W1 x + b1); q = q * (drop1 >= .5) * 2
  q = sigmoid(W2 q + b2); q = q * (drop2 >= .5) * 2
  q = sigmoid(W3 q + b3); q = sigmoid(W4 q + b4); out = W5 q + b5

Sharding: batch 64 -> 8 cores x 8 samples (pure data parallel).

v2 design (per-sample software pipeline; v1 did two 4-sample half-waves):
  - All big per-sample blobs (w1x_j, wm_j) ride the SYNC HWDGE ring in
    exact consumption order (one HWDGE queue saturates ~360-400 GB/s;
    v1's scalar-ring-first assumption was inverted by ACT_TABLE_LOADs
    delaying the scalar ring ~2.6us). ScalarE does no DMA arming at all
    so its FIFO is pure ACT work. GpSimd SWDGE carries the small blobs
    (sc, cb, wb).
  - Per-sample waves: iteration j runs L1_j, L2_{j-1}, L3_{j-2}, L4_{j-3}
    on PE with ACT/STT of older samples interleaved on ScalarE/DVE. This
    keeps the PE continuously busy (no >3.4us idle gaps -> HAM stays at
    K=8/8 after warmup; v1 oscillated and ran most MMs at 1.2 GHz) and
    drains the backlog so only the last sample's serial chain trails the
    final DMA byte.
  - One-hot matrices are generated on-chip by DVE memsets (v1 shipped
    them from HBM inside cb). b3 is folded into ACT3's per-partition
    bias operand (L3 output partitions = couts), dropping those bias MMs.
  - L4/L5/out split into two 4-sample chains: chain a completes mid-DMA,
    only chain b is tail.
  - fp8/bf16 numerics identical to v1: fp8e4 weights pre-scaled by pow2
    factors compensated via ACT scale operands; masks {0,2} exact in fp8.
  - PSUM: 8 banks: p1 x2 (j%2), p2 x2 (j%2), p3 x1 (2 slots), p4 x2
    (quads), p5 x1 (both chains + PE warmup group).
"""

import time

import ml_dtypes
import numpy as np

import concourse.tile as tile
from concourse import bacc, mybir
from concourse.bass_utils import run_bass_kernel_spmd

N_CORES = 8
S = 8            # samples per core
HW = 64
F8NP = ml_dtypes.float8_e4m3
BFNP = ml_dtypes.bfloat16

BF16 = mybir.dt.bfloat16
F8 = mybir.dt.float8e4
F32 = mybir.dt.float32
SIG = mybir.ActivationFunctionType.Sigmoid
MULT = mybir.AluOpType.mult

# --- w1x blob columns (fp8): w1T chunks (k0..7, m0..3)*128, then x chunks ---
X_OFF = 4096                    # 8 chunks x 64
W1X_COLS = 4608
# --- wm blob columns (fp8): w2T (k0..3, m0..1)*128, w3T (k0..1)*128, masks ---
W3_OFF = 1024
MK1_OFF = 1280                  # mask1 [128, 4*64]
MK2_OFF = 1536                  # mask2 [128, 2*64]
WM_COLS = 1664
# --- wb blob (bf16) [128, S*65]: per sample w4T(64) + w5col(1) ---
WB_PER = 65
WB_COLS = S * WB_PER
# --- cb blob (bf16) [8, 2690]: one-hot + per-sample bias lhsT tiles ---
OH_OFF = 0                      # rows 0-7: block-diagonal one-hot [8, 512]
B1_OFF = 512                    # rows 0-3: b1_j [4,128] at cols j*128
B2_OFF = 1536                   # rows 0-1: b2_j [2,128]
B4_OFF = 2560                   # rows 0-3: b4 quad a [4,64], quad b [4,64]
B5_OFF = 2688                   # rows 0-3: b5 quad a [4,1], quad b [4,1]
CB_COLS = 2690
# --- sc blob (f32) [128, 11]: 3 act scales + 8 per-sample b3 columns ---
SC_COLS = 11

_COMPILED = None
LAST_RESULT = None


def _build():
    nc = bacc.Bacc(target_bir_lowering=False)
    w1x_d = nc.declare_dram_parameter("w1x", [S, 128, W1X_COLS], F8, isOutput=False)
    wm_d = nc.declare_dram_parameter("wm", [S, 128, WM_COLS], F8, isOutput=False)
    cb_d = nc.declare_dram_parameter("cb", [8, CB_COLS], BF16, isOutput=False)
    sc_d = nc.declare_dram_parameter("sc", [128, SC_COLS], F32, isOutput=False)
    wb_d = nc.declare_dram_parameter("wb", [128, WB_COLS], BF16, isOutput=False)
    out_d = nc.declare_dram_parameter("out", [1, S * HW], F32, isOutput=True)

    with tile.TileContext(nc) as tc:
        with (
            tc.tile_pool(name="sbuf", bufs=1) as sb,
            tc.tile_pool(name="psum", bufs=1, space="PSUM") as ps,
        ):
            scT = sb.tile([128, SC_COLS], F32, tag="scT")
            cbT = sb.tile([8, CB_COLS], BF16, tag="cbT")
            wbT = sb.tile([128, WB_COLS], BF16, tag="wbT")
            warm = sb.tile([128, HW], BF16, tag="warm")

            w1xT = [sb.tile([128, W1X_COLS], F8, tag=f"w1x{j}", name=f"w1x{j}")
                    for j in range(S)]
            wmT = [sb.tile([128, WM_COLS], F8, tag=f"wm{j}", name=f"wm{j}")
                   for j in range(S)]
            q1 = [sb.tile([128, 4, HW], BF16, tag=f"q1_{j}", name=f"q1_{j}")
                  for j in range(S)]
            q2 = [sb.tile([128, 2, HW], BF16, tag=f"q2_{j}", name=f"q2_{j}")
                  for j in range(S)]
            q3 = [sb.tile([128, HW], BF16, tag=f"q3_{j}", name=f"q3_{j}")
                  for j in range(S)]
            q4 = [sb.tile([64, 4, HW], BF16, tag=f"q4_{h}", name=f"q4_{h}")
                  for h in range(2)]
            outs = [sb.tile([1, 4, HW], F32, tag=f"out_{h}", name=f"out_{h}")
                    for h in range(2)]

            # Small blobs ride the scalar HWDGE ring (otherwise idle for
            # DMA): gpsimd SWDGE would contend for the same SDMA engines
            # as the sync ring and straggle the w1x0 completion sem by
            # ~2us (cb is an 8-partition blob -> 1-2 engines only).
            nc.scalar.dma_start(out=cbT[:], in_=cb_d[:, :])
            nc.scalar.dma_start(out=scT[:], in_=sc_d[:, :])
            nc.scalar.dma_start(out=wbT[:], in_=wb_d[:, :])

            # DVE preamble: warmup operand.
            nc.vector.memset(warm[:], 0.0)
            ohT = cbT[0:8, OH_OFF:OH_OFF + 512]

            # All big blobs on the sync HWDGE ring in consumption order.
            for j in range(S):
                nc.sync.dma_start(out=w1xT[j][:], in_=w1x_d[j, :, :])
                nc.sync.dma_start(out=wmT[j][:], in_=wm_d[j, :, :])

            # PSUM: 8 banks (padded to a full bank each).
            p1 = [ps.tile([128, 4, HW], F32, tag=f"p1_{s}", name=f"p1_{s}",
                          padded_shape=[128, 4, 2 * HW]) for s in range(2)]
            p2 = [ps.tile([128, 2, HW], F32, tag=f"p2_{s}", name=f"p2_{s}",
                          padded_shape=[128, 2, 4 * HW]) for s in range(2)]
            p3 = ps.tile([128, 2, HW], F32, tag="p3",
                         padded_shape=[128, 2, 4 * HW])
            p4 = [ps.tile([64, 4, HW], F32, tag=f"p4_{h}", name=f"p4_{h}",
                          padded_shape=[64, 4, 2 * HW]) for h in range(2)]
            p5 = ps.tile([1, 2, 4, HW], F32, tag="p5")

            # PE warmup: one long accumulation group keeps HAM from
            # throttling while the first weight DMA is in flight. 136 MMs
            # span ~3.4us cold + ~2us warm, bridging until w1x0 lands
            # (~12.3us): any PE idle >~0.5us inside a free-running 3.4us
            # HAM window re-throttles the clock to 1.2 GHz, and the
            # un-throttle threshold (a ~full busy window) is never met
            # again in DMA-paced steady state.
            NWARM = 136
            for i in range(NWARM):
                nc.tensor.matmul(p5[:, 0, 0, :], warm[:, 0:1], warm[:],
                                 start=(i == 0), stop=(i == NWARM - 1),
                                 skip_group_check=True)

            def filler(n):
                # Short PE busy-burst between early, DMA-starved samples
                # (same HAM-density purpose as the warmup). Writes the
                # chain-b L5 region; L5b's start=True clears it later.
                for i in range(n):
                    nc.tensor.matmul(p5[:, 1, 0, :], warm[:, 0:1], warm[:],
                                     start=(i == 0), stop=(i == n - 1),
                                     skip_group_check=True)

            sc1 = scT[:, 0:1]
            sc2 = scT[:, 1:2]
            sc3 = scT[:, 2:3]

            def l1(j):
                pt = p1[j % 2]
                wt = w1xT[j]
                for m in range(4):
                    for k in range(8):
                        nc.tensor.matmul(
                            pt[:, m, :],
                            wt[:, (k * 4 + m) * 128:(k * 4 + m + 1) * 128],
                            wt[:, X_OFF + k * HW:X_OFF + (k + 1) * HW],
                            start=(m == 0 and k == 0), stop=False,
                            skip_group_check=True)
                nc.tensor.matmul(
                    pt[:, :, :],
                    cbT[0:4, B1_OFF + j * 128:B1_OFF + (j + 1) * 128],
                    ohT[0:4, 0:256],
                    start=False, stop=True, skip_group_check=True)

            def act1(j):
                nc.scalar.activation(q1[j][:, :, :], p1[j % 2][:, :, :],
                                     SIG, scale=sc1)
                nc.vector.scalar_tensor_tensor(
                    out=q1[j][:, :, :],
                    in0=wmT[j][:, MK1_OFF:MK1_OFF + 256].rearrange(
                        "p (m t) -> p m t", m=4),
                    scalar=1.0, in1=q1[j][:, :, :], op0=MULT, op1=MULT)

            def l2(j):
                pt = p2[j % 2]
                wt = wmT[j]
                for m in range(2):
                    for k in range(4):
                        nc.tensor.matmul(
                            pt[:, m, :],
                            wt[:, (k * 2 + m) * 128:(k * 2 + m + 1) * 128],
                            q1[j][:, k, :],
                            start=(m == 0 and k == 0), stop=False,
                            skip_group_check=True)
                nc.tensor.matmul(
                    pt[:, :, :],
                    cbT[0:2, B2_OFF + j * 128:B2_OFF + (j + 1) * 128],
                    ohT[0:2, 0:128],
                    start=False, stop=True, skip_group_check=True)

            def act2(j):
                nc.scalar.activation(q2[j][:, :, :], p2[j % 2][:, :, :],
                                     SIG, scale=sc2)
                nc.vector.scalar_tensor_tensor(
                    out=q2[j][:, :, :],
                    in0=wmT[j][:, MK2_OFF:MK2_OFF + 128].rearrange(
                        "p (m t) -> p m t", m=2),
                    scalar=1.0, in1=q2[j][:, :, :], op0=MULT, op1=MULT)

            def l3(j):
                for k in range(2):
                    nc.tensor.matmul(
                        p3[:, j % 2, :],
                        wmT[j][:, W3_OFF + k * 128:W3_OFF + (k + 1) * 128],
                        q2[j][:, k, :],
                        start=(k == 0), stop=(k == 1), skip_group_check=True)

            def act3(j):
                # b3 folded into the per-partition bias operand.
                nc.scalar.activation(q3[j][:, :], p3[:, j % 2, :], SIG,
                                     bias=scT[:, 3 + j:4 + j], scale=sc3)

            def l4(j):
                h, i = j // 4, j % 4
                nc.tensor.matmul(
                    p4[h][:, i, :], wbT[:, j * WB_PER:j * WB_PER + 64],
                    q3[j][:, :], start=(i == 0), stop=False,
                    skip_group_check=True)
                if i == 3:
                    nc.tensor.matmul(
                        p4[h][:, :, :],
                        cbT[0:4, B4_OFF + h * 64:B4_OFF + (h + 1) * 64],
                        ohT[0:4, 0:256],
                        start=False, stop=True, skip_group_check=True)
                    nc.scalar.activation(q4[h][:, :, :], p4[h][:, :, :], SIG)

            def l5(h):
                for i in range(4):
                    j = 4 * h + i
                    nc.tensor.matmul(
                        p5[:, h, i, :],
                        wbT[0:64, j * WB_PER + 64:j * WB_PER + 65],
                        q4[h][:, i, :], start=(i == 0), stop=False,
                        skip_group_check=True)
                nc.tensor.matmul(
                    p5[:, h, :, :], cbT[0:4, B5_OFF + h:B5_OFF + h + 1],
                    ohT[0:4, 0:256],
                    start=False, stop=True, skip_group_check=True)
                nc.vector.tensor_scalar_mul(outs[h][:, :, :],
                                            p5[:, h, :, :], 1.0)
                nc.sync.dma_start(
                    out=out_d[0:1, h * 256:(h + 1) * 256],
                    in_=outs[h].rearrange("p a b -> p (a b)"))

            # Deep software pipeline: every cross-engine dependency is at
            # least one full iteration (~2.2us) old by the time its PE
            # consumer can issue, so scheduler reorderings can't stall
            # the PE FIFO (v2 used 1-iteration offsets and lost ~730ns
            # per iteration to ACT/STT head-of-line waits).
            FILL = [40, 32, 24, 16, 12, 8, 0, 0, 0, 0, 0, 0]
            for it in range(12):
                if it < 8:
                    l1(it)
                if 0 <= it - 1 < 8:
                    act1(it - 1)
                if 0 <= it - 2 < 8:
                    l2(it - 2)
                    act2(it - 2)
                if 0 <= it - 3 < 8:
                    l3(it - 3)
                    act3(it - 3)
                if 0 <= it - 4 < 8:
                    l4(it - 4)
                if it - 4 == 3:
                    l5(0)       # chain a completes mid-DMA
                if FILL[it]:
                    filler(FILL[it])
            l5(1)
    nc.compile()
    return nc


def _pow2_scale(a, cap=224.0):
    m = float(np.abs(a).max())
    if m == 0.0:
        return 1.0
    return float(2.0 ** np.floor(np.log2(cap / m)))


def _pack(x, w1, b1, w2, b2, w3, b3, w4, b4, w5, b5, drop1, drop2):
    """Build per-sample w1x/wm blobs; return bias/scale data."""
    B = x.shape[0]
    f4 = np.float32
    x3 = np.ascontiguousarray(x.reshape(B, 1024, HW), dtype=f4)
    w1m = w1.reshape(B, 512, 1024).astype(f4, copy=False)
    w2m = w2.reshape(B, 256, 512).astype(f4, copy=False)
    w3m = w3.reshape(B, 128, 256).astype(f4, copy=False)
    w4m = w4.reshape(B, 64, 128).astype(f4, copy=False)
    w5m = w5.reshape(B, 64).astype(f4, copy=False)

    sx = _pow2_scale(x3)
    s1 = _pow2_scale(w1m)
    s2 = _pow2_scale(w2m)
    s3 = _pow2_scale(w3m)

    def chunkT(wT, nk, nm):  # [B, cin, cout] -> [B, 128, nk*nm*128]
        Bn, cin, cout = wT.shape
        return np.ascontiguousarray(
            wT.reshape(Bn, nk, 128, nm, 128).transpose(0, 2, 1, 3, 4)
        ).reshape(Bn, 128, nk * nm * 128)

    w1T = chunkT(np.swapaxes(w1m, 1, 2) * s1, 8, 4)
    xc = np.ascontiguousarray(
        x3.reshape(B, 8, 128, HW).transpose(0, 2, 1, 3)).reshape(B, 128, 512) * sx
    w1x = np.concatenate([w1T, xc], axis=2).astype(F8NP)

    w2T = chunkT(np.swapaxes(w2m, 1, 2) * s2, 4, 2)
    w3T = chunkT(np.swapaxes(w3m, 1, 2) * s3, 2, 1)
    m1 = (drop1.reshape(B, 512, HW) >= np.float32(0.5)).astype(f4) * f4(2.0)
    m1 = np.ascontiguousarray(
        m1.reshape(B, 4, 128, HW).transpose(0, 2, 1, 3)).reshape(B, 128, 256)
    m2 = (drop2.reshape(B, 256, HW) >= np.float32(0.5)).astype(f4) * f4(2.0)
    m2 = np.ascontiguousarray(
        m2.reshape(B, 2, 128, HW).transpose(0, 2, 1, 3)).reshape(B, 128, 128)
    wm = np.concatenate([w2T, w3T, m1, m2], axis=2).astype(F8NP)

    wb = np.zeros((B, 128, WB_PER), f4)
    wb[:, :, 0:64] = np.swapaxes(w4m, 1, 2)
    wb[:, :64, 64] = w5m

    b1s = b1.astype(f4) * f4(s1 * sx)
    b2s = b2.astype(f4) * f4(s2)
    scales = (1.0 / (s1 * sx), 1.0 / s2, 1.0 / s3)
    return w1x, wm, wb, (b1s, b2s, b3.astype(f4), b4.astype(f4),
                         b5.reshape(B).astype(f4)), scales


def kernel(**inputs):
    global _COMPILED, LAST_RESULT
    if _COMPILED is None:
        _COMPILED = _build()
    nc = _COMPILED

    w1x, wm, wb, (b1s, b2s, b3f, b4f, b5f), scales = _pack(
        **{k: np.asarray(v) for k, v in inputs.items()})

    in_maps = []
    for c in range(N_CORES):
        sl = slice(c * S, (c + 1) * S)
        wbc = wb[sl].transpose(1, 0, 2).reshape(128, S * WB_PER)

        cb = np.zeros((8, CB_COLS), np.float32)
        for r in range(8):
            cb[r, OH_OFF + r * HW:OH_OFF + (r + 1) * HW] = 1.0
        for j in range(S):
            g = c * S + j
            cb[0:4, B1_OFF + j * 128:B1_OFF + (j + 1) * 128] = \
                b1s[g].reshape(4, 128)
            cb[0:2, B2_OFF + j * 128:B2_OFF + (j + 1) * 128] = \
                b2s[g].reshape(2, 128)
        cb[0:4, B4_OFF:B4_OFF + 64] = b4f[c * S:c * S + 4]
        cb[0:4, B4_OFF + 64:B4_OFF + 128] = b4f[c * S + 4:c * S + 8]
        cb[0:4, B5_OFF] = b5f[c * S:c * S + 4]
        cb[0:4, B5_OFF + 1] = b5f[c * S + 4:c * S + 8]

        scc = np.empty((128, SC_COLS), np.float32)
        scc[:, 0] = scales[0]
        scc[:, 1] = scales[1]
        scc[:, 2] = scales[2]
        for j in range(S):
            scc[:, 3 + j] = b3f[c * S + j]

        in_maps.append({
            "w1x": np.ascontiguousarray(w1x[sl]),
            "wm": np.ascontiguousarray(wm[sl]),
            "cb": cb.astype(BFNP),
            "sc": scc,
            "wb": np.ascontiguousarray(wbc).astype(BFNP),
        })

    res = None
    for attempt in range(3):
        try:
            res = run_bass_kernel_spmd(nc, in_maps, core_ids=list(range(N_CORES)))
            break
        except Exception:
            if attempt == 2:
                raise
            time.sleep(20)
            try:  # best-effort device reconnect after NRT_EXEC_UNIT_UNRECOVERABLE
                import jax
                jax.clear_caches()
                import jax.extend.backend as _jeb
                _jeb.clear_backends()
            except Exception:
                pass
    LAST_RESULT = res
    outs = [np.asarray(res.results[c]["out"]).reshape(S, 8, 8)
            for c in range(N_CORES)]
    return np.concatenate(outs, axis=0).astype(np.float32)


# revision 16
# speedup vs baseline: 1.1680x; 1.0990x over previous
"""Trainium2 Bass kernel for AFCNet (per-sample 1x1-conv MLP), 8-core data parallel.

Network per sample b (dims 1024 -> 512 -> 256 -> 128 -> 64 -> 1, HW=64):
  q = sigmoid(W1 x + b1); q = q * (drop1 >= .5) * 2
  q = sigmoid(W2 q + b2); q = q * (drop2 >= .5) * 2
  q = sigmoid(W3 q + b3); q = sigmoid(W4 q + b4); out = W5 q + b5

Sharding: batch 64 -> 8 cores x 8 samples (pure data parallel).

v2 design (per-sample software pipeline; v1 did two 4-sample half-waves):
  - All big per-sample blobs (w1x_j, wm_j) ride the SYNC HWDGE ring in
    exact consumption order (one HWDGE queue saturates ~360-400 GB/s;
    v1's scalar-ring-first assumption was inverted by ACT_TABLE_LOADs
    delaying the scalar ring ~2.6us). ScalarE does no DMA arming at all
    so its FIFO is pure ACT work. GpSimd SWDGE carries the small blobs
    (sc, cb, wb).
  - Per-sample waves: iteration j runs L1_j, L2_{j-1}, L3_{j-2}, L4_{j-3}
    on PE with ACT/STT of older samples interleaved on ScalarE/DVE. This
    keeps the PE continuously busy (no >3.4us idle gaps -> HAM stays at
    K=8/8 after warmup; v1 oscillated and ran most MMs at 1.2 GHz) and
    drains the backlog so only the last sample's serial chain trails the
    final DMA byte.
  - One-hot matrices are generated on-chip by DVE memsets (v1 shipped
    them from HBM inside cb). b3 is folded into ACT3's per-partition
    bias operand (L3 output partitions = couts), dropping those bias MMs.
  - L4/L5/out split into two 4-sample chains: chain a completes mid-DMA,
    only chain b is tail.
  - fp8/bf16 numerics identical to v1: fp8e4 weights pre-scaled by pow2
    factors compensated via ACT scale operands; masks {0,2} exact in fp8.
  - PSUM: 8 banks: p1 x2 (j%2), p2 x2 (j%2), p3 x1 (2 slots), p4 x2
    (quads), p5 x1 (both chains + PE warmup group).
"""

import time

import ml_dtypes
import numpy as np

import concourse.tile as tile
from concourse import bacc, mybir
from concourse.bass_utils import run_bass_kernel_spmd

N_CORES = 8
S = 8            # samples per core
HW = 64
F8NP = ml_dtypes.float8_e4m3
BFNP = ml_dtypes.bfloat16

BF16 = mybir.dt.bfloat16
F8 = mybir.dt.float8e4
F32 = mybir.dt.float32
SIG = mybir.ActivationFunctionType.Sigmoid
MULT = mybir.AluOpType.mult

# --- w1x blob columns (fp8): w1T chunks (k0..7, m0..3)*128, then x chunks ---
X_OFF = 4096                    # 8 chunks x 64
W1X_COLS = 4608
# --- wm blob columns (fp8): w2T (k0..3, m0..1)*128, w3T (k0..1)*128, masks ---
W3_OFF = 1024
MK1_OFF = 1280                  # mask1 [128, 4*64]
MK2_OFF = 1536                  # mask2 [128, 2*64]
WM_COLS = 1664
# --- wb blob (bf16) [128, S*65]: per sample w4T(64) + w5col(1) ---
WB_PER = 65
WB_COLS = S * WB_PER
# --- cb blob (bf16) [8, 2690]: one-hot + per-sample bias lhsT tiles ---
OH_OFF = 0                      # rows 0-7: block-diagonal one-hot [8, 512]
B1_OFF = 512                    # rows 0-3: b1_j [4,128] at cols j*128
B2_OFF = 1536                   # rows 0-1: b2_j [2,128]
B4_OFF = 2560                   # rows 0-3: b4 quad a [4,64], quad b [4,64]
B5_OFF = 2688                   # rows 0-3: b5 quad a [4,1], quad b [4,1]
CB_COLS = 2690
# --- sc blob (f32) [128, 11]: 3 act scales + 8 per-sample b3 columns ---
SC_COLS = 11

_COMPILED = None
LAST_RESULT = None


def _build():
    nc = bacc.Bacc(target_bir_lowering=False)
    w1x_d = nc.declare_dram_parameter("w1x", [S, 128, W1X_COLS], F8, isOutput=False)
    wm_d = nc.declare_dram_parameter("wm", [S, 128, WM_COLS], F8, isOutput=False)
    cb_d = nc.declare_dram_parameter("cb", [8, CB_COLS], BF16, isOutput=False)
    sc_d = nc.declare_dram_parameter("sc", [128, SC_COLS], F32, isOutput=False)
    wb_d = nc.declare_dram_parameter("wb", [128, WB_COLS], BF16, isOutput=False)
    out_d = nc.declare_dram_parameter("out", [1, S * HW], F32, isOutput=True)

    with tile.TileContext(nc) as tc:
        with (
            tc.tile_pool(name="sbuf", bufs=1) as sb,
            tc.tile_pool(name="psum", bufs=1, space="PSUM") as ps,
        ):
            scT = sb.tile([128, SC_COLS], F32, tag="scT")
            cbT = sb.tile([8, CB_COLS], BF16, tag="cbT")
            wbT = sb.tile([128, WB_COLS], BF16, tag="wbT")
            warm = sb.tile([128, 512], BF16, tag="warm")

            w1xT = [sb.tile([128, W1X_COLS], F8, tag=f"w1x{j}", name=f"w1x{j}")
                    for j in range(S)]
            wmT = [sb.tile([128, WM_COLS], F8, tag=f"wm{j}", name=f"wm{j}")
                   for j in range(S)]
            q1 = [sb.tile([128, 4, HW], BF16, tag=f"q1_{j}", name=f"q1_{j}")
                  for j in range(S)]
            q2 = [sb.tile([128, 2, HW], BF16, tag=f"q2_{j}", name=f"q2_{j}")
                  for j in range(S)]
            q3 = [sb.tile([128, HW], BF16, tag=f"q3_{j}", name=f"q3_{j}")
                  for j in range(S)]
            q4 = [sb.tile([64, 4, HW], BF16, tag=f"q4_{h}", name=f"q4_{h}")
                  for h in range(2)]
            outs = [sb.tile([1, 4, HW], F32, tag=f"out_{h}", name=f"out_{h}")
                    for h in range(2)]

            # Small blobs ride the scalar HWDGE ring (otherwise idle for
            # DMA): gpsimd SWDGE would contend for the same SDMA engines
            # as the sync ring and straggle the w1x0 completion sem by
            # ~2us (cb is an 8-partition blob -> 1-2 engines only).
            nc.scalar.dma_start(out=cbT[:], in_=cb_d[:, :])
            nc.scalar.dma_start(out=scT[:], in_=sc_d[:, :])
            nc.scalar.dma_start(out=wbT[:], in_=wb_d[:, :])

            # DVE preamble: warmup operand.
            nc.vector.memset(warm[:], 0.0)
            ohT = cbT[0:8, OH_OFF:OH_OFF + 512]

            # All big blobs on the sync HWDGE ring in consumption order.
            for j in range(S):
                nc.sync.dma_start(out=w1xT[j][:], in_=w1x_d[j, :, :])
                nc.sync.dma_start(out=wmT[j][:], in_=wm_d[j, :, :])

            # PSUM: 8 banks (padded to a full bank each).
            p1 = [ps.tile([128, 4, HW], F32, tag=f"p1_{s}", name=f"p1_{s}",
                          padded_shape=[128, 4, 2 * HW]) for s in range(2)]
            p2 = [ps.tile([128, 2, HW], F32, tag=f"p2_{s}", name=f"p2_{s}",
                          padded_shape=[128, 2, 4 * HW]) for s in range(2)]
            p3 = ps.tile([128, 2, HW], F32, tag="p3",
                         padded_shape=[128, 2, 4 * HW])
            p4 = [ps.tile([64, 4, HW], F32, tag=f"p4_{h}", name=f"p4_{h}",
                          padded_shape=[64, 4, 2 * HW]) for h in range(2)]
            p5 = ps.tile([1, 2, 4, HW], F32, tag="p5")

            # PE warmup: one long accumulation group keeps HAM from
            # throttling while the first weight DMA is in flight,
            # bridging until w1x0 lands (~12.3us): any PE idle >~0.5us
            # inside a free-running 3.4us HAM window re-throttles the
            # clock to 1.2 GHz, and the un-throttle threshold (a ~full
            # busy window) is never met again in DMA-paced steady state.
            # N=512 moving operands keep the instruction count small --
            # the tensor stream must stay within few 16KiB IRAM blocks
            # or sequential ifetch stalls the PE ~1.4us per boundary
            # while the data DMA saturates HBM.
            p5flat = p5.rearrange("p a b t -> p (a b t)")
            NWARM = 14
            for i in range(NWARM):
                nc.tensor.matmul(p5flat[:, :], warm[:, 0:1], warm[:],
                                 start=(i == 0), stop=(i == NWARM - 1),
                                 skip_group_check=True)

            def filler(n):
                # Short PE busy-burst between early, DMA-starved samples
                # (same HAM-density purpose as the warmup). p5 is
                # cleared later by the L5 groups' start=True.
                for i in range(n):
                    nc.tensor.matmul(p5flat[:, :], warm[:, 0:1], warm[:],
                                     start=(i == 0), stop=(i == n - 1),
                                     skip_group_check=True)

            sc1 = scT[:, 0:1]
            sc2 = scT[:, 1:2]
            sc3 = scT[:, 2:3]

            def l1(j):
                pt = p1[j % 2]
                wt = w1xT[j]
                for m in range(4):
                    for k in range(8):
                        nc.tensor.matmul(
                            pt[:, m, :],
                            wt[:, (k * 4 + m) * 128:(k * 4 + m + 1) * 128],
                            wt[:, X_OFF + k * HW:X_OFF + (k + 1) * HW],
                            start=(m == 0 and k == 0), stop=False,
                            skip_group_check=True)
                nc.tensor.matmul(
                    pt[:, :, :],
                    cbT[0:4, B1_OFF + j * 128:B1_OFF + (j + 1) * 128],
                    ohT[0:4, 0:256],
                    start=False, stop=True, skip_group_check=True)

            def act1(j):
                nc.scalar.activation(q1[j][:, :, :], p1[j % 2][:, :, :],
                                     SIG, scale=sc1)
                nc.vector.scalar_tensor_tensor(
                    out=q1[j][:, :, :],
                    in0=wmT[j][:, MK1_OFF:MK1_OFF + 256].rearrange(
                        "p (m t) -> p m t", m=4),
                    scalar=1.0, in1=q1[j][:, :, :], op0=MULT, op1=MULT)

            def l2(j):
                pt = p2[j % 2]
                wt = wmT[j]
                for m in range(2):
                    for k in range(4):
                        nc.tensor.matmul(
                            pt[:, m, :],
                            wt[:, (k * 2 + m) * 128:(k * 2 + m + 1) * 128],
                            q1[j][:, k, :],
                            start=(m == 0 and k == 0), stop=False,
                            skip_group_check=True)
                nc.tensor.matmul(
                    pt[:, :, :],
                    cbT[0:2, B2_OFF + j * 128:B2_OFF + (j + 1) * 128],
                    ohT[0:2, 0:128],
                    start=False, stop=True, skip_group_check=True)

            def act2(j):
                nc.scalar.activation(q2[j][:, :, :], p2[j % 2][:, :, :],
                                     SIG, scale=sc2)
                nc.vector.scalar_tensor_tensor(
                    out=q2[j][:, :, :],
                    in0=wmT[j][:, MK2_OFF:MK2_OFF + 128].rearrange(
                        "p (m t) -> p m t", m=2),
                    scalar=1.0, in1=q2[j][:, :, :], op0=MULT, op1=MULT)

            def l3(j):
                for k in range(2):
                    nc.tensor.matmul(
                        p3[:, j % 2, :],
                        wmT[j][:, W3_OFF + k * 128:W3_OFF + (k + 1) * 128],
                        q2[j][:, k, :],
                        start=(k == 0), stop=(k == 1), skip_group_check=True)

            def act3(j):
                # b3 folded into the per-partition bias operand.
                nc.scalar.activation(q3[j][:, :], p3[:, j % 2, :], SIG,
                                     bias=scT[:, 3 + j:4 + j], scale=sc3)

            def l4(j):
                h, i = j // 4, j % 4
                nc.tensor.matmul(
                    p4[h][:, i, :], wbT[:, j * WB_PER:j * WB_PER + 64],
                    q3[j][:, :], start=(i == 0), stop=False,
                    skip_group_check=True)
                if i == 3:
                    nc.tensor.matmul(
                        p4[h][:, :, :],
                        cbT[0:4, B4_OFF + h * 64:B4_OFF + (h + 1) * 64],
                        ohT[0:4, 0:256],
                        start=False, stop=True, skip_group_check=True)
                    nc.scalar.activation(q4[h][:, :, :], p4[h][:, :, :], SIG)

            def l5(h):
                for i in range(4):
                    j = 4 * h + i
                    nc.tensor.matmul(
                        p5[:, h, i, :],
                        wbT[0:64, j * WB_PER + 64:j * WB_PER + 65],
                        q4[h][:, i, :], start=(i == 0), stop=False,
                        skip_group_check=True)
                nc.tensor.matmul(
                    p5[:, h, :, :], cbT[0:4, B5_OFF + h:B5_OFF + h + 1],
                    ohT[0:4, 0:256],
                    start=False, stop=True, skip_group_check=True)
                nc.vector.tensor_scalar_mul(outs[h][:, :, :],
                                            p5[:, h, :, :], 1.0)
                nc.sync.dma_start(
                    out=out_d[0:1, h * 256:(h + 1) * 256],
                    in_=outs[h].rearrange("p a b -> p (a b)"))

            # Deep software pipeline: every cross-engine dependency is at
            # least one full iteration (~2.2us) old by the time its PE
            # consumer can issue, so scheduler reorderings can't stall
            # the PE FIFO (v2 used 1-iteration offsets and lost ~730ns
            # per iteration to ACT/STT head-of-line waits).
            FILL = [5, 4, 3, 2, 2, 1, 0, 0, 0, 0, 0, 0]
            for it in range(12):
                if it < 8:
                    l1(it)
                if 0 <= it - 1 < 8:
                    act1(it - 1)
                if 0 <= it - 2 < 8:
                    l2(it - 2)
                    act2(it - 2)
                if 0 <= it - 3 < 8:
                    l3(it - 3)
                    act3(it - 3)
                if 0 <= it - 4 < 8:
                    l4(it - 4)
                if it == 9:
                    l5(0)       # chain a: deps (ACT4a at it7) 2 its old
                if FILL[it]:
                    filler(FILL[it])
            l5(1)
    nc.compile()
    return nc


def _pow2_scale(a, cap=224.0):
    m = float(np.abs(a).max())
    if m == 0.0:
        return 1.0
    return float(2.0 ** np.floor(np.log2(cap / m)))


def _pack(x, w1, b1, w2, b2, w3, b3, w4, b4, w5, b5, drop1, drop2):
    """Build per-sample w1x/wm blobs; return bias/scale data."""
    B = x.shape[0]
    f4 = np.float32
    x3 = np.ascontiguousarray(x.reshape(B, 1024, HW), dtype=f4)
    w1m = w1.reshape(B, 512, 1024).astype(f4, copy=False)
    w2m = w2.reshape(B, 256, 512).astype(f4, copy=False)
    w3m = w3.reshape(B, 128, 256).astype(f4, copy=False)
    w4m = w4.reshape(B, 64, 128).astype(f4, copy=False)
    w5m = w5.reshape(B, 64).astype(f4, copy=False)

    sx = _pow2_scale(x3)
    s1 = _pow2_scale(w1m)
    s2 = _pow2_scale(w2m)
    s3 = _pow2_scale(w3m)

    def chunkT(wT, nk, nm):  # [B, cin, cout] -> [B, 128, nk*nm*128]
        Bn, cin, cout = wT.shape
        return np.ascontiguousarray(
            wT.reshape(Bn, nk, 128, nm, 128).transpose(0, 2, 1, 3, 4)
        ).reshape(Bn, 128, nk * nm * 128)

    w1T = chunkT(np.swapaxes(w1m, 1, 2) * s1, 8, 4)
    xc = np.ascontiguousarray(
        x3.reshape(B, 8, 128, HW).transpose(0, 2, 1, 3)).reshape(B, 128, 512) * sx
    w1x = np.concatenate([w1T, xc], axis=2).astype(F8NP)

    w2T = chunkT(np.swapaxes(w2m, 1, 2) * s2, 4, 2)
    w3T = chunkT(np.swapaxes(w3m, 1, 2) * s3, 2, 1)
    m1 = (drop1.reshape(B, 512, HW) >= np.float32(0.5)).astype(f4) * f4(2.0)
    m1 = np.ascontiguousarray(
        m1.reshape(B, 4, 128, HW).transpose(0, 2, 1, 3)).reshape(B, 128, 256)
    m2 = (drop2.reshape(B, 256, HW) >= np.float32(0.5)).astype(f4) * f4(2.0)
    m2 = np.ascontiguousarray(
        m2.reshape(B, 2, 128, HW).transpose(0, 2, 1, 3)).reshape(B, 128, 128)
    wm = np.concatenate([w2T, w3T, m1, m2], axis=2).astype(F8NP)

    wb = np.zeros((B, 128, WB_PER), f4)
    wb[:, :, 0:64] = np.swapaxes(w4m, 1, 2)
    wb[:, :64, 64] = w5m

    b1s = b1.astype(f4) * f4(s1 * sx)
    b2s = b2.astype(f4) * f4(s2)
    scales = (1.0 / (s1 * sx), 1.0 / s2, 1.0 / s3)
    return w1x, wm, wb, (b1s, b2s, b3.astype(f4), b4.astype(f4),
                         b5.reshape(B).astype(f4)), scales


def kernel(**inputs):
    global _COMPILED, LAST_RESULT
    if _COMPILED is None:
        _COMPILED = _build()
    nc = _COMPILED

    w1x, wm, wb, (b1s, b2s, b3f, b4f, b5f), scales = _pack(
        **{k: np.asarray(v) for k, v in inputs.items()})

    in_maps = []
    for c in range(N_CORES):
        sl = slice(c * S, (c + 1) * S)
        wbc = wb[sl].transpose(1, 0, 2).reshape(128, S * WB_PER)

        cb = np.zeros((8, CB_COLS), np.float32)
        for r in range(8):
            cb[r, OH_OFF + r * HW:OH_OFF + (r + 1) * HW] = 1.0
        for j in range(S):
            g = c * S + j
            cb[0:4, B1_OFF + j * 128:B1_OFF + (j + 1) * 128] = \
                b1s[g].reshape(4, 128)
            cb[0:2, B2_OFF + j * 128:B2_OFF + (j + 1) * 128] = \
                b2s[g].reshape(2, 128)
        cb[0:4, B4_OFF:B4_OFF + 64] = b4f[c * S:c * S + 4]
        cb[0:4, B4_OFF + 64:B4_OFF + 128] = b4f[c * S + 4:c * S + 8]
        cb[0:4, B5_OFF] = b5f[c * S:c * S + 4]
        cb[0:4, B5_OFF + 1] = b5f[c * S + 4:c * S + 8]

        scc = np.empty((128, SC_COLS), np.float32)
        scc[:, 0] = scales[0]
        scc[:, 1] = scales[1]
        scc[:, 2] = scales[2]
        for j in range(S):
            scc[:, 3 + j] = b3f[c * S + j]

        in_maps.append({
            "w1x": np.ascontiguousarray(w1x[sl]),
            "wm": np.ascontiguousarray(wm[sl]),
            "cb": cb.astype(BFNP),
            "sc": scc,
            "wb": np.ascontiguousarray(wbc).astype(BFNP),
        })

    res = None
    for attempt in range(3):
        try:
            res = run_bass_kernel_spmd(nc, in_maps, core_ids=list(range(N_CORES)))
            break
        except Exception:
            if attempt == 2:
                raise
            time.sleep(20)
            try:  # best-effort device reconnect after NRT_EXEC_UNIT_UNRECOVERABLE
                import jax
                jax.clear_caches()
                import jax.extend.backend as _jeb
                _jeb.clear_backends()
            except Exception:
                pass
    LAST_RESULT = res
    outs = [np.asarray(res.results[c]["out"]).reshape(S, 8, 8)
            for c in range(N_CORES)]
    return np.concatenate(outs, axis=0).astype(np.float32)


# revision 23
# speedup vs baseline: 1.1939x; 1.0222x over previous
"""Trainium2 Bass kernel for AFCNet (per-sample 1x1-conv MLP), 8-core data parallel.

Network per sample b (dims 1024 -> 512 -> 256 -> 128 -> 64 -> 1, HW=64):
  q = sigmoid(W1 x + b1); q = q * (drop1 >= .5) * 2
  q = sigmoid(W2 q + b2); q = q * (drop2 >= .5) * 2
  q = sigmoid(W3 q + b3); q = sigmoid(W4 q + b4); out = W5 q + b5

Sharding: batch 64 -> 8 cores x 8 samples (pure data parallel).

v2 design (per-sample software pipeline; v1 did two 4-sample half-waves):
  - All big per-sample blobs (w1x_j, wm_j) ride the SYNC HWDGE ring in
    exact consumption order (one HWDGE queue saturates ~360-400 GB/s;
    v1's scalar-ring-first assumption was inverted by ACT_TABLE_LOADs
    delaying the scalar ring ~2.6us). ScalarE does no DMA arming at all
    so its FIFO is pure ACT work. GpSimd SWDGE carries the small blobs
    (sc, cb, wb).
  - Per-sample waves: iteration j runs L1_j, L2_{j-1}, L3_{j-2}, L4_{j-3}
    on PE with ACT/STT of older samples interleaved on ScalarE/DVE. This
    keeps the PE continuously busy (no >3.4us idle gaps -> HAM stays at
    K=8/8 after warmup; v1 oscillated and ran most MMs at 1.2 GHz) and
    drains the backlog so only the last sample's serial chain trails the
    final DMA byte.
  - One-hot matrices are generated on-chip by DVE memsets (v1 shipped
    them from HBM inside cb). b3 is folded into ACT3's per-partition
    bias operand (L3 output partitions = couts), dropping those bias MMs.
  - L4/L5/out split into two 4-sample chains: chain a completes mid-DMA,
    only chain b is tail.
  - fp8/bf16 numerics identical to v1: fp8e4 weights pre-scaled by pow2
    factors compensated via ACT scale operands; masks {0,2} exact in fp8.
  - PSUM: 8 banks: p1 x2 (j%2), p2 x2 (j%2), p3 x1 (2 slots), p4 x2
    (quads), p5 x1 (both chains + PE warmup group).
"""

import time

import ml_dtypes
import numpy as np

import concourse.tile as tile
from concourse import bacc, mybir
from concourse.bass_utils import run_bass_kernel_spmd

N_CORES = 8
S = 8            # samples per core
HW = 64
F8NP = ml_dtypes.float8_e4m3
BFNP = ml_dtypes.bfloat16

BF16 = mybir.dt.bfloat16
F8 = mybir.dt.float8e4
F32 = mybir.dt.float32
SIG = mybir.ActivationFunctionType.Sigmoid
MULT = mybir.AluOpType.mult

# --- w1x blob columns (fp8): x chunks first, then w1T chunks (k,m)*128 ---
X_OFF = 0                       # 8 chunks x 64
W_OFF = 512                     # chunk (k*4+m) at W_OFF + (k*4+m)*128
W1X_COLS = 4608
SPLIT_A = 2560                  # [x | k0-3] / [k4-7] split point
SPLIT_JS = (0, 1, 2, 7)         # samples whose w1x ships as two DMAs
# --- wm blob columns (fp8): w2T (k0..3, m0..1)*128, w3T (k0..1)*128, masks ---
W3_OFF = 1024
MK1_OFF = 1280                  # mask1 [128, 4*64]
MK2_OFF = 1536                  # mask2 [128, 2*64]
WM_COLS = 1664
# --- wb blob (bf16) [128, S*65]: per sample w4T(64) + w5col(1) ---
WB_PER = 65
WB_COLS = S * WB_PER
# --- cb blob (bf16) [8, 2690]: one-hot + per-sample bias lhsT tiles ---
OH_OFF = 0                      # rows 0-7: block-diagonal one-hot [8, 512]
B1_OFF = 512                    # rows 0-3: b1_j [4,128] at cols j*128
B2_OFF = 1536                   # rows 0-1: b2_j [2,128]
B4_OFF = 2560                   # rows 0-3: b4 quad a [4,64], quad b [4,64]
B5_OFF = 2688                   # rows 0-3: b5 quad a [4,1], quad b [4,1]
CB_COLS = 2690
# --- sc blob (f32) [128, 11]: 3 act scales + 8 per-sample b3 columns ---
SC_COLS = 11

_COMPILED = None
LAST_RESULT = None


def _build():
    nc = bacc.Bacc(target_bir_lowering=False)
    w1x_d = nc.declare_dram_parameter("w1x", [S, 128, W1X_COLS], F8, isOutput=False)
    wm_d = nc.declare_dram_parameter("wm", [S, 128, WM_COLS], F8, isOutput=False)
    cb_d = nc.declare_dram_parameter("cb", [8, CB_COLS], BF16, isOutput=False)
    sc_d = nc.declare_dram_parameter("sc", [128, SC_COLS], F32, isOutput=False)
    wb_d = nc.declare_dram_parameter("wb", [128, WB_COLS], BF16, isOutput=False)
    out_d = nc.declare_dram_parameter("out", [1, S * HW], F32, isOutput=True)

    with tile.TileContext(nc) as tc:
        with (
            tc.tile_pool(name="sbuf", bufs=1) as sb,
            tc.tile_pool(name="psum", bufs=1, space="PSUM") as ps,
        ):
            scT = sb.tile([128, SC_COLS], F32, tag="scT")
            cbT = sb.tile([8, CB_COLS], BF16, tag="cbT")
            wbT = sb.tile([128, WB_COLS], BF16, tag="wbT")
            warm = sb.tile([128, 512], BF16, tag="warm")

            w1xT = []
            for j in range(S):
                if j in SPLIT_JS:
                    w1xT.append((
                        sb.tile([128, SPLIT_A], F8, tag=f"w1xa{j}",
                                name=f"w1xa{j}"),
                        sb.tile([128, W1X_COLS - SPLIT_A], F8,
                                tag=f"w1xb{j}", name=f"w1xb{j}"),
                    ))
                else:
                    w1xT.append(sb.tile([128, W1X_COLS], F8, tag=f"w1x{j}",
                                        name=f"w1x{j}"))
            wmT = [sb.tile([128, WM_COLS], F8, tag=f"wm{j}", name=f"wm{j}")
                   for j in range(S)]
            q1 = [sb.tile([128, 4, HW], BF16, tag=f"q1_{j}", name=f"q1_{j}")
                  for j in range(S)]
            q2 = [sb.tile([128, 2, HW], BF16, tag=f"q2_{j}", name=f"q2_{j}")
                  for j in range(S)]
            q3 = [sb.tile([128, HW], BF16, tag=f"q3_{j}", name=f"q3_{j}")
                  for j in range(S)]
            q4 = [sb.tile([64, 4, HW], BF16, tag=f"q4_{h}", name=f"q4_{h}")
                  for h in range(2)]
            outs = [sb.tile([1, 4, HW], F32, tag=f"out_{h}", name=f"out_{h}")
                    for h in range(2)]

            # Small blobs ride the scalar HWDGE ring (otherwise idle for
            # DMA): gpsimd SWDGE would contend for the same SDMA engines
            # as the sync ring and straggle the w1x0 completion sem by
            # ~2us (cb is an 8-partition blob -> 1-2 engines only).
            nc.scalar.dma_start(out=cbT[:], in_=cb_d[:, :])
            nc.scalar.dma_start(out=scT[:], in_=sc_d[:, :])
            nc.scalar.dma_start(out=wbT[:], in_=wb_d[:, :])

            # DVE preamble: warmup operand.
            nc.vector.memset(warm[:], 0.0)
            ohT = cbT[0:8, OH_OFF:OH_OFF + 512]

            # All big blobs on the sync HWDGE ring in consumption order.
            # Early (and last) samples ship w1x as two DMAs so L1_j's
            # first half can start ~1us before the full blob lands.
            for j in range(S):
                if j in SPLIT_JS:
                    nc.sync.dma_start(out=w1xT[j][0][:],
                                      in_=w1x_d[j, :, 0:SPLIT_A])
                    nc.sync.dma_start(out=w1xT[j][1][:],
                                      in_=w1x_d[j, :, SPLIT_A:W1X_COLS])
                else:
                    nc.sync.dma_start(out=w1xT[j][:], in_=w1x_d[j, :, :])
                nc.sync.dma_start(out=wmT[j][:], in_=wm_d[j, :, :])

            # PSUM: 8 banks (padded to a full bank each).
            p1 = [ps.tile([128, 4, HW], F32, tag=f"p1_{s}", name=f"p1_{s}",
                          padded_shape=[128, 4, 2 * HW]) for s in range(2)]
            p2 = [ps.tile([128, 2, HW], F32, tag=f"p2_{s}", name=f"p2_{s}",
                          padded_shape=[128, 2, 4 * HW]) for s in range(2)]
            p3 = ps.tile([128, 2, HW], F32, tag="p3",
                         padded_shape=[128, 2, 4 * HW])
            p4 = [ps.tile([64, 4, HW], F32, tag=f"p4_{h}", name=f"p4_{h}",
                          padded_shape=[64, 4, 2 * HW]) for h in range(2)]
            p5 = ps.tile([1, 2, 4, HW], F32, tag="p5")

            # PE warmup: one long accumulation group keeps HAM from
            # throttling while the first weight DMA is in flight,
            # bridging until w1x0 lands (~12.3us): any PE idle >~0.5us
            # inside a free-running 3.4us HAM window re-throttles the
            # clock to 1.2 GHz, and the un-throttle threshold (a ~full
            # busy window) is never met again in DMA-paced steady state.
            # N=512 moving operands keep the instruction count small --
            # the tensor stream must stay within few 16KiB IRAM blocks
            # or sequential ifetch stalls the PE ~1.4us per boundary
            # while the data DMA saturates HBM.
            p5flat = p5.rearrange("p a b t -> p (a b t)")
            NWARM = 7
            for i in range(NWARM):
                nc.tensor.matmul(p5flat[:, :], warm[:, 0:1], warm[:],
                                 start=(i == 0), stop=(i == NWARM - 1),
                                 skip_group_check=True)

            def filler(n):
                # Short PE busy-burst between early, DMA-starved samples
                # (same HAM-density purpose as the warmup). p5 is
                # cleared later by the L5 groups' start=True.
                for i in range(n):
                    nc.tensor.matmul(p5flat[:, :], warm[:, 0:1], warm[:],
                                     start=(i == 0), stop=(i == n - 1),
                                     skip_group_check=True)

            sc1 = scT[:, 0:1]
            sc2 = scT[:, 1:2]
            sc3 = scT[:, 2:3]

            def l1(j):
                pt = p1[j % 2]
                wt = w1xT[j]
                split = j in SPLIT_JS

                def w_chunk(c):  # lhsT chunk c = k*4+m
                    if not split:
                        return wt[:, W_OFF + c * 128:W_OFF + (c + 1) * 128]
                    if W_OFF + (c + 1) * 128 <= SPLIT_A:
                        return wt[0][:, W_OFF + c * 128:W_OFF + (c + 1) * 128]
                    off = W_OFF + c * 128 - SPLIT_A
                    return wt[1][:, off:off + 128]

                xa = wt[0] if split else wt
                # k-halves outer so the first 16 MMs only need the A blob
                for kh in range(2):
                    for m in range(4):
                        for k in range(4 * kh, 4 * kh + 4):
                            nc.tensor.matmul(
                                pt[:, m, :],
                                w_chunk(k * 4 + m),
                                xa[:, X_OFF + k * HW:X_OFF + (k + 1) * HW],
                                start=(kh == 0 and m == 0 and k == 0),
                                stop=False, skip_group_check=True)
                nc.tensor.matmul(
                    pt[:, :, :],
                    cbT[0:4, B1_OFF + j * 128:B1_OFF + (j + 1) * 128],
                    ohT[0:4, 0:256],
                    start=False, stop=True, skip_group_check=True)

            def act1(j):
                nc.scalar.activation(q1[j][:, :, :], p1[j % 2][:, :, :],
                                     SIG, scale=sc1)
                nc.vector.scalar_tensor_tensor(
                    out=q1[j][:, :, :],
                    in0=wmT[j][:, MK1_OFF:MK1_OFF + 256].rearrange(
                        "p (m t) -> p m t", m=4),
                    scalar=1.0, in1=q1[j][:, :, :], op0=MULT, op1=MULT)

            def l2(j):
                pt = p2[j % 2]
                wt = wmT[j]
                for m in range(2):
                    for k in range(4):
                        nc.tensor.matmul(
                            pt[:, m, :],
                            wt[:, (k * 2 + m) * 128:(k * 2 + m + 1) * 128],
                            q1[j][:, k, :],
                            start=(m == 0 and k == 0), stop=False,
                            skip_group_check=True)
                nc.tensor.matmul(
                    pt[:, :, :],
                    cbT[0:2, B2_OFF + j * 128:B2_OFF + (j + 1) * 128],
                    ohT[0:2, 0:128],
                    start=False, stop=True, skip_group_check=True)

            def act2(j):
                nc.scalar.activation(q2[j][:, :, :], p2[j % 2][:, :, :],
                                     SIG, scale=sc2)
                nc.vector.scalar_tensor_tensor(
                    out=q2[j][:, :, :],
                    in0=wmT[j][:, MK2_OFF:MK2_OFF + 128].rearrange(
                        "p (m t) -> p m t", m=2),
                    scalar=1.0, in1=q2[j][:, :, :], op0=MULT, op1=MULT)

            def l3(j):
                for k in range(2):
                    nc.tensor.matmul(
                        p3[:, j % 2, :],
                        wmT[j][:, W3_OFF + k * 128:W3_OFF + (k + 1) * 128],
                        q2[j][:, k, :],
                        start=(k == 0), stop=(k == 1), skip_group_check=True)

            def act3(j):
                # b3 folded into the per-partition bias operand.
                nc.scalar.activation(q3[j][:, :], p3[:, j % 2, :], SIG,
                                     bias=scT[:, 3 + j:4 + j], scale=sc3)

            def l4(j):
                h, i = j // 4, j % 4
                nc.tensor.matmul(
                    p4[h][:, i, :], wbT[:, j * WB_PER:j * WB_PER + 64],
                    q3[j][:, :], start=(i == 0), stop=False,
                    skip_group_check=True)
                if i == 3:
                    nc.tensor.matmul(
                        p4[h][:, :, :],
                        cbT[0:4, B4_OFF + h * 64:B4_OFF + (h + 1) * 64],
                        ohT[0:4, 0:256],
                        start=False, stop=True, skip_group_check=True)
                    nc.scalar.activation(q4[h][:, :, :], p4[h][:, :, :], SIG)

            def l5(h):
                for i in range(4):
                    j = 4 * h + i
                    nc.tensor.matmul(
                        p5[:, h, i, :],
                        wbT[0:64, j * WB_PER + 64:j * WB_PER + 65],
                        q4[h][:, i, :], start=(i == 0), stop=False,
                        skip_group_check=True)
                nc.tensor.matmul(
                    p5[:, h, :, :], cbT[0:4, B5_OFF + h:B5_OFF + h + 1],
                    ohT[0:4, 0:256],
                    start=False, stop=True, skip_group_check=True)
                nc.vector.tensor_scalar_mul(outs[h][:, :, :],
                                            p5[:, h, :, :], 1.0)
                nc.sync.dma_start(
                    out=out_d[0:1, h * 256:(h + 1) * 256],
                    in_=outs[h].rearrange("p a b -> p (a b)"))

            # Deep software pipeline: every cross-engine dependency is at
            # least one full iteration (~2.2us) old by the time its PE
            # consumer can issue, so scheduler reorderings can't stall
            # the PE FIFO (v2 used 1-iteration offsets and lost ~730ns
            # per iteration to ACT/STT head-of-line waits).
            FILL = [3, 3, 2, 2, 1, 1, 0, 0, 0, 0, 0, 0]
            for it in range(12):
                if it < 8:
                    l1(it)
                if 0 <= it - 1 < 8:
                    act1(it - 1)
                if 0 <= it - 2 < 8:
                    l2(it - 2)
                    act2(it - 2)
                if 0 <= it - 3 < 8:
                    l3(it - 3)
                    act3(it - 3)
                if 0 <= it - 4 < 8:
                    l4(it - 4)
                if it == 9:
                    l5(0)       # chain a: deps (ACT4a at it7) 2 its old
                if FILL[it]:
                    filler(FILL[it])
            l5(1)
    nc.compile()
    return nc


def _pow2_scale(a, cap=224.0):
    m = float(np.abs(a).max())
    if m == 0.0:
        return 1.0
    return float(2.0 ** np.floor(np.log2(cap / m)))


def _pack(x, w1, b1, w2, b2, w3, b3, w4, b4, w5, b5, drop1, drop2):
    """Build per-sample w1x/wm blobs; return bias/scale data."""
    B = x.shape[0]
    f4 = np.float32
    x3 = np.ascontiguousarray(x.reshape(B, 1024, HW), dtype=f4)
    w1m = w1.reshape(B, 512, 1024).astype(f4, copy=False)
    w2m = w2.reshape(B, 256, 512).astype(f4, copy=False)
    w3m = w3.reshape(B, 128, 256).astype(f4, copy=False)
    w4m = w4.reshape(B, 64, 128).astype(f4, copy=False)
    w5m = w5.reshape(B, 64).astype(f4, copy=False)

    sx = _pow2_scale(x3)
    s1 = _pow2_scale(w1m)
    s2 = _pow2_scale(w2m)
    s3 = _pow2_scale(w3m)

    def chunkT(wT, nk, nm):  # [B, cin, cout] -> [B, 128, nk*nm*128]
        Bn, cin, cout = wT.shape
        return np.ascontiguousarray(
            wT.reshape(Bn, nk, 128, nm, 128).transpose(0, 2, 1, 3, 4)
        ).reshape(Bn, 128, nk * nm * 128)

    w1T = chunkT(np.swapaxes(w1m, 1, 2) * s1, 8, 4)
    xc = np.ascontiguousarray(
        x3.reshape(B, 8, 128, HW).transpose(0, 2, 1, 3)).reshape(B, 128, 512) * sx
    w1x = np.concatenate([xc, w1T], axis=2).astype(F8NP)

    w2T = chunkT(np.swapaxes(w2m, 1, 2) * s2, 4, 2)
    w3T = chunkT(np.swapaxes(w3m, 1, 2) * s3, 2, 1)
    m1 = (drop1.reshape(B, 512, HW) >= np.float32(0.5)).astype(f4) * f4(2.0)
    m1 = np.ascontiguousarray(
        m1.reshape(B, 4, 128, HW).transpose(0, 2, 1, 3)).reshape(B, 128, 256)
    m2 = (drop2.reshape(B, 256, HW) >= np.float32(0.5)).astype(f4) * f4(2.0)
    m2 = np.ascontiguousarray(
        m2.reshape(B, 2, 128, HW).transpose(0, 2, 1, 3)).reshape(B, 128, 128)
    wm = np.concatenate([w2T, w3T, m1, m2], axis=2).astype(F8NP)

    wb = np.zeros((B, 128, WB_PER), f4)
    wb[:, :, 0:64] = np.swapaxes(w4m, 1, 2)
    wb[:, :64, 64] = w5m

    b1s = b1.astype(f4) * f4(s1 * sx)
    b2s = b2.astype(f4) * f4(s2)
    scales = (1.0 / (s1 * sx), 1.0 / s2, 1.0 / s3)
    return w1x, wm, wb, (b1s, b2s, b3.astype(f4), b4.astype(f4),
                         b5.reshape(B).astype(f4)), scales


def kernel(**inputs):
    global _COMPILED, LAST_RESULT
    if _COMPILED is None:
        _COMPILED = _build()
    nc = _COMPILED

    w1x, wm, wb, (b1s, b2s, b3f, b4f, b5f), scales = _pack(
        **{k: np.asarray(v) for k, v in inputs.items()})

    in_maps = []
    for c in range(N_CORES):
        sl = slice(c * S, (c + 1) * S)
        wbc = wb[sl].transpose(1, 0, 2).reshape(128, S * WB_PER)

        cb = np.zeros((8, CB_COLS), np.float32)
        for r in range(8):
            cb[r, OH_OFF + r * HW:OH_OFF + (r + 1) * HW] = 1.0
        for j in range(S):
            g = c * S + j
            cb[0:4, B1_OFF + j * 128:B1_OFF + (j + 1) * 128] = \
                b1s[g].reshape(4, 128)
            cb[0:2, B2_OFF + j * 128:B2_OFF + (j + 1) * 128] = \
                b2s[g].reshape(2, 128)
        cb[0:4, B4_OFF:B4_OFF + 64] = b4f[c * S:c * S + 4]
        cb[0:4, B4_OFF + 64:B4_OFF + 128] = b4f[c * S + 4:c * S + 8]
        cb[0:4, B5_OFF] = b5f[c * S:c * S + 4]
        cb[0:4, B5_OFF + 1] = b5f[c * S + 4:c * S + 8]

        scc = np.empty((128, SC_COLS), np.float32)
        scc[:, 0] = scales[0]
        scc[:, 1] = scales[1]
        scc[:, 2] = scales[2]
        for j in range(S):
            scc[:, 3 + j] = b3f[c * S + j]

        in_maps.append({
            "w1x": np.ascontiguousarray(w1x[sl]),
            "wm": np.ascontiguousarray(wm[sl]),
            "cb": cb.astype(BFNP),
            "sc": scc,
            "wb": np.ascontiguousarray(wbc).astype(BFNP),
        })

    res = None
    for attempt in range(3):
        try:
            res = run_bass_kernel_spmd(nc, in_maps, core_ids=list(range(N_CORES)))
            break
        except Exception:
            if attempt == 2:
                raise
            time.sleep(20)
            try:  # best-effort device reconnect after NRT_EXEC_UNIT_UNRECOVERABLE
                import jax
                jax.clear_caches()
                import jax.extend.backend as _jeb
                _jeb.clear_backends()
            except Exception:
                pass
    LAST_RESULT = res
    outs = [np.asarray(res.results[c]["out"]).reshape(S, 8, 8)
            for c in range(N_CORES)]
    return np.concatenate(outs, axis=0).astype(np.float32)


# revision 30
# speedup vs baseline: 1.2335x; 1.0332x over previous
"""Trainium2 Bass kernel for AFCNet (per-sample 1x1-conv MLP), 8-core data parallel.

Network per sample b (dims 1024 -> 512 -> 256 -> 128 -> 64 -> 1, HW=64):
  q = sigmoid(W1 x + b1); q = q * (drop1 >= .5) * 2
  q = sigmoid(W2 q + b2); q = q * (drop2 >= .5) * 2
  q = sigmoid(W3 q + b3); q = sigmoid(W4 q + b4); out = W5 q + b5

Sharding: batch 64 -> 8 cores x 8 samples (pure data parallel).

v2 design (per-sample software pipeline; v1 did two 4-sample half-waves):
  - All big per-sample blobs (w1x_j, wm_j) ride the SYNC HWDGE ring in
    exact consumption order (one HWDGE queue saturates ~360-400 GB/s;
    v1's scalar-ring-first assumption was inverted by ACT_TABLE_LOADs
    delaying the scalar ring ~2.6us). ScalarE does no DMA arming at all
    so its FIFO is pure ACT work. GpSimd SWDGE carries the small blobs
    (sc, cb, wb).
  - Per-sample waves: iteration j runs L1_j, L2_{j-1}, L3_{j-2}, L4_{j-3}
    on PE with ACT/STT of older samples interleaved on ScalarE/DVE. This
    keeps the PE continuously busy (no >3.4us idle gaps -> HAM stays at
    K=8/8 after warmup; v1 oscillated and ran most MMs at 1.2 GHz) and
    drains the backlog so only the last sample's serial chain trails the
    final DMA byte.
  - One-hot matrices are generated on-chip by DVE memsets (v1 shipped
    them from HBM inside cb). b3 is folded into ACT3's per-partition
    bias operand (L3 output partitions = couts), dropping those bias MMs.
  - L4/L5/out split into two 4-sample chains: chain a completes mid-DMA,
    only chain b is tail.
  - fp8/bf16 numerics identical to v1: fp8e4 weights pre-scaled by pow2
    factors compensated via ACT scale operands; masks {0,2} exact in fp8.
  - PSUM: 8 banks: p1 x2 (j%2), p2 x2 (j%2), p3 x1 (2 slots), p4 x2
    (quads), p5 x1 (both chains + PE warmup group).
"""

import time

import ml_dtypes
import numpy as np

import concourse.tile as tile
from concourse import bacc, mybir
from concourse.bass_utils import run_bass_kernel_spmd
from concourse.tile_rust import add_dep_helper

N_CORES = 8
S = 8            # samples per core
HW = 64
F8NP = ml_dtypes.float8_e4m3
BFNP = ml_dtypes.bfloat16

BF16 = mybir.dt.bfloat16
F8 = mybir.dt.float8e4
F32 = mybir.dt.float32
SIG = mybir.ActivationFunctionType.Sigmoid
MULT = mybir.AluOpType.mult

# --- w1x blob columns (fp8): x chunks first, then w1T chunks (k,m)*128 ---
X_OFF = 0                       # 8 chunks x 64
W_OFF = 512                     # chunk (k*4+m) at W_OFF + (k*4+m)*128
W1X_COLS = 4608
SPLIT_A = 2560                  # [x | k0-3] / [k4-7] split point
SPLIT_JS = (0, 1, 2, 7)         # samples whose w1x ships as two DMAs
# --- wm blob columns (fp8): w2T (k0..3, m0..1)*128, w3T (k0..1)*128, masks ---
W3_OFF = 1024
MK1_OFF = 1280                  # mask1 [128, 4*64]
MK2_OFF = 1536                  # mask2 [128, 2*64]
WM_COLS = 1664
# --- wb blob (bf16) [128, S*65]: per sample w4T(64) + w5col(1) ---
WB_PER = 65
WB_COLS = S * WB_PER
# --- cb blob (bf16) [8, 2690]: one-hot + per-sample bias lhsT tiles ---
OH_OFF = 0                      # rows 0-7: block-diagonal one-hot [8, 512]
B1_OFF = 512                    # rows 0-3: b1_j [4,128] at cols j*128
B2_OFF = 1536                   # rows 0-1: b2_j [2,128]
B4_OFF = 2560                   # rows 0-3: b4 quad a [4,64], quad b [4,64]
B5_OFF = 2688                   # rows 0-3: b5 quad a [4,1], quad b [4,1]
CB_COLS = 2690
# --- sc blob (f32) [128, 11]: 3 act scales + 8 per-sample b3 columns ---
SC_COLS = 11

_COMPILED = None
LAST_RESULT = None


def _build():
    nc = bacc.Bacc(target_bir_lowering=False)
    w1x_d = nc.declare_dram_parameter("w1x", [S, 128, W1X_COLS], F8, isOutput=False)
    wm_d = nc.declare_dram_parameter("wm", [S, 128, WM_COLS], F8, isOutput=False)
    cb_d = nc.declare_dram_parameter("cb", [8, CB_COLS], BF16, isOutput=False)
    sc_d = nc.declare_dram_parameter("sc", [128, SC_COLS], F32, isOutput=False)
    wb_d = nc.declare_dram_parameter("wb", [128, WB_COLS], BF16, isOutput=False)
    out_d = nc.declare_dram_parameter("out", [1, S * HW], F32, isOutput=True)

    with tile.TileContext(nc) as tc:
        with (
            tc.tile_pool(name="sbuf", bufs=1) as sb,
            tc.tile_pool(name="psum", bufs=1, space="PSUM") as ps,
        ):
            scT = sb.tile([128, SC_COLS], F32, tag="scT")
            cbT = sb.tile([8, CB_COLS], BF16, tag="cbT")
            wbT = sb.tile([128, WB_COLS], BF16, tag="wbT")
            warm = sb.tile([128, 512], BF16, tag="warm")

            w1xT = []
            for j in range(S):
                if j in SPLIT_JS:
                    w1xT.append((
                        sb.tile([128, SPLIT_A], F8, tag=f"w1xa{j}",
                                name=f"w1xa{j}"),
                        sb.tile([128, W1X_COLS - SPLIT_A], F8,
                                tag=f"w1xb{j}", name=f"w1xb{j}"),
                    ))
                else:
                    w1xT.append(sb.tile([128, W1X_COLS], F8, tag=f"w1x{j}",
                                        name=f"w1x{j}"))
            wmT = [sb.tile([128, WM_COLS], F8, tag=f"wm{j}", name=f"wm{j}")
                   for j in range(S)]
            q1 = [sb.tile([128, 4, HW], BF16, tag=f"q1_{j}", name=f"q1_{j}")
                  for j in range(S)]
            q2 = [sb.tile([128, 2, HW], BF16, tag=f"q2_{j}", name=f"q2_{j}")
                  for j in range(S)]
            q3 = [sb.tile([128, HW], BF16, tag=f"q3_{j}", name=f"q3_{j}")
                  for j in range(S)]
            q4 = [sb.tile([64, 4, HW], BF16, tag=f"q4_{h}", name=f"q4_{h}")
                  for h in range(2)]
            outs = [sb.tile([1, 4, HW], F32, tag=f"out_{h}", name=f"out_{h}")
                    for h in range(2)]

            # Small blobs ride the scalar HWDGE ring (otherwise idle for
            # DMA): gpsimd SWDGE would contend for the same SDMA engines
            # as the sync ring and straggle the w1x0 completion sem by
            # ~2us (cb is an 8-partition blob -> 1-2 engines only).
            nc.scalar.dma_start(out=cbT[:], in_=cb_d[:, :])
            nc.scalar.dma_start(out=scT[:], in_=sc_d[:, :])
            nc.scalar.dma_start(out=wbT[:], in_=wb_d[:, :])

            # DVE preamble: warmup operand.
            nc.vector.memset(warm[:], 0.0)
            ohT = cbT[0:8, OH_OFF:OH_OFF + 512]

            # All big blobs on the sync HWDGE ring in consumption order.
            # Early (and last) samples ship w1x as two DMAs so L1_j's
            # first half can start ~1us before the full blob lands.
            for j in range(S):
                if j in SPLIT_JS:
                    nc.sync.dma_start(out=w1xT[j][0][:],
                                      in_=w1x_d[j, :, 0:SPLIT_A])
                    nc.sync.dma_start(out=w1xT[j][1][:],
                                      in_=w1x_d[j, :, SPLIT_A:W1X_COLS])
                else:
                    nc.sync.dma_start(out=w1xT[j][:], in_=w1x_d[j, :, :])
                nc.sync.dma_start(out=wmT[j][:], in_=wm_d[j, :, :])

            # PSUM: 8 banks (padded to a full bank each).
            p1 = [ps.tile([128, 4, HW], F32, tag=f"p1_{s}", name=f"p1_{s}",
                          padded_shape=[128, 4, 2 * HW]) for s in range(2)]
            p2 = [ps.tile([128, 2, HW], F32, tag=f"p2_{s}", name=f"p2_{s}",
                          padded_shape=[128, 2, 4 * HW]) for s in range(2)]
            p3 = ps.tile([128, 2, HW], F32, tag="p3",
                         padded_shape=[128, 2, 4 * HW])
            p4 = [ps.tile([64, 4, HW], F32, tag=f"p4_{h}", name=f"p4_{h}",
                          padded_shape=[64, 4, 2 * HW]) for h in range(2)]
            p5 = ps.tile([1, 2, 4, HW], F32, tag="p5")


            # The Tile scheduler orders each engine's stream by its cost
            # model's readiness estimates, which puts L2_j right before
            # L1_{j+1} and serializes the ACT1->STT1->L2 chain into the
            # L1 cadence (~3us/sample instead of the 2.24us DMA pace).
            # Chain each engine's ops in emission order to force the
            # software-pipeline schedule.
            _last = {}

            def _ordered(key, binst):
                prev = _last.get(key)
                if prev is not None:
                    add_dep_helper(binst.ins, prev.ins, sync=False,
                                   reason="forced engine order")
                _last[key] = binst
                return binst

            def pe_mm(*args, **kw):
                return _ordered("pe", nc.tensor.matmul(*args, **kw))

            def sc_act(*args, **kw):
                return _ordered("act", nc.scalar.activation(*args, **kw))

            def ve_op(binst):
                return _ordered("dve", binst)

            # PE warmup: one long accumulation group keeps HAM from
            # throttling while the first weight DMA is in flight,
            # bridging until w1x0 lands (~12.3us): any PE idle >~0.5us
            # inside a free-running 3.4us HAM window re-throttles the
            # clock to 1.2 GHz, and the un-throttle threshold (a ~full
            # busy window) is never met again in DMA-paced steady state.
            # N=512 moving operands keep the instruction count small --
            # the tensor stream must stay within few 16KiB IRAM blocks
            # or sequential ifetch stalls the PE ~1.4us per boundary
            # while the data DMA saturates HBM.
            p5flat = p5.rearrange("p a b t -> p (a b t)")
            NWARM = 7
            for i in range(NWARM):
                pe_mm(p5flat[:, :], warm[:, 0:1], warm[:],
                                 start=(i == 0), stop=(i == NWARM - 1),
                                 skip_group_check=True)

            def filler(n):
                # Short PE busy-burst between early, DMA-starved samples
                # (same HAM-density purpose as the warmup). p5 is
                # cleared later by the L5 groups' start=True.
                for i in range(n):
                    pe_mm(p5flat[:, :], warm[:, 0:1], warm[:],
                                     start=(i == 0), stop=(i == n - 1),
                                     skip_group_check=True)

            sc1 = scT[:, 0:1]
            sc2 = scT[:, 1:2]
            sc3 = scT[:, 2:3]


            def l1(j):
                pt = p1[j % 2]
                wt = w1xT[j]
                split = j in SPLIT_JS

                def w_chunk(c):  # lhsT chunk c = k*4+m
                    if not split:
                        return wt[:, W_OFF + c * 128:W_OFF + (c + 1) * 128]
                    if W_OFF + (c + 1) * 128 <= SPLIT_A:
                        return wt[0][:, W_OFF + c * 128:W_OFF + (c + 1) * 128]
                    off = W_OFF + c * 128 - SPLIT_A
                    return wt[1][:, off:off + 128]

                xa = wt[0] if split else wt
                # k-halves outer so the first 16 MMs only need the A blob
                for kh in range(2):
                    for m in range(4):
                        for k in range(4 * kh, 4 * kh + 4):
                            pe_mm(
                                pt[:, m, :],
                                w_chunk(k * 4 + m),
                                xa[:, X_OFF + k * HW:X_OFF + (k + 1) * HW],
                                start=(kh == 0 and m == 0 and k == 0),
                                stop=False, skip_group_check=True)
                pe_mm(
                    pt[:, :, :],
                    cbT[0:4, B1_OFF + j * 128:B1_OFF + (j + 1) * 128],
                    ohT[0:4, 0:256],
                    start=False, stop=True, skip_group_check=True)

            def act1(j):
                sc_act(q1[j][:, :, :], p1[j % 2][:, :, :],
                                     SIG, scale=sc1)
                ve_op(nc.vector.scalar_tensor_tensor(
                    out=q1[j][:, :, :],
                    in0=wmT[j][:, MK1_OFF:MK1_OFF + 256].rearrange(
                        "p (m t) -> p m t", m=4),
                    scalar=1.0, in1=q1[j][:, :, :], op0=MULT, op1=MULT))

            def l2(j):
                pt = p2[j % 2]
                wt = wmT[j]
                for m in range(2):
                    for k in range(4):
                        pe_mm(
                            pt[:, m, :],
                            wt[:, (k * 2 + m) * 128:(k * 2 + m + 1) * 128],
                            q1[j][:, k, :],
                            start=(m == 0 and k == 0), stop=False,
                            skip_group_check=True)
                pe_mm(
                    pt[:, :, :],
                    cbT[0:2, B2_OFF + j * 128:B2_OFF + (j + 1) * 128],
                    ohT[0:2, 0:128],
                    start=False, stop=True, skip_group_check=True)

            def act2(j):
                sc_act(q2[j][:, :, :], p2[j % 2][:, :, :],
                                     SIG, scale=sc2)
                ve_op(nc.vector.scalar_tensor_tensor(
                    out=q2[j][:, :, :],
                    in0=wmT[j][:, MK2_OFF:MK2_OFF + 128].rearrange(
                        "p (m t) -> p m t", m=2),
                    scalar=1.0, in1=q2[j][:, :, :], op0=MULT, op1=MULT))

            def l3(j):
                for k in range(2):
                    pe_mm(
                        p3[:, j % 2, :],
                        wmT[j][:, W3_OFF + k * 128:W3_OFF + (k + 1) * 128],
                        q2[j][:, k, :],
                        start=(k == 0), stop=(k == 1), skip_group_check=True)

            def act3(j):
                # b3 folded into the per-partition bias operand.
                sc_act(q3[j][:, :], p3[:, j % 2, :], SIG,
                                     bias=scT[:, 3 + j:4 + j], scale=sc3)

            def l4(j):
                h, i = j // 4, j % 4
                pe_mm(
                    p4[h][:, i, :], wbT[:, j * WB_PER:j * WB_PER + 64],
                    q3[j][:, :], start=(i == 0), stop=False,
                    skip_group_check=True)
                if i == 3:
                    pe_mm(
                        p4[h][:, :, :],
                        cbT[0:4, B4_OFF + h * 64:B4_OFF + (h + 1) * 64],
                        ohT[0:4, 0:256],
                        start=False, stop=True, skip_group_check=True)
                    sc_act(q4[h][:, :, :], p4[h][:, :, :], SIG)

            def l5(h):
                for i in range(4):
                    j = 4 * h + i
                    pe_mm(
                        p5[:, h, i, :],
                        wbT[0:64, j * WB_PER + 64:j * WB_PER + 65],
                        q4[h][:, i, :], start=(i == 0), stop=False,
                        skip_group_check=True)
                pe_mm(
                    p5[:, h, :, :], cbT[0:4, B5_OFF + h:B5_OFF + h + 1],
                    ohT[0:4, 0:256],
                    start=False, stop=True, skip_group_check=True)
                ve_op(nc.vector.tensor_scalar_mul(outs[h][:, :, :],
                                                  p5[:, h, :, :], 1.0))
                nc.sync.dma_start(
                    out=out_d[0:1, h * 256:(h + 1) * 256],
                    in_=outs[h].rearrange("p a b -> p (a b)"))

            # Deep software pipeline: every cross-engine dependency is at
            # least one full iteration (~2.2us) old by the time its PE
            # consumer can issue, so scheduler reorderings can't stall
            # the PE FIFO (v2 used 1-iteration offsets and lost ~730ns
            # per iteration to ACT/STT head-of-line waits).
            FILL = [3, 3, 2, 2, 1, 1, 0, 0, 0, 0, 0, 0]
            for it in range(12):
                if it < 8:
                    l1(it)
                if 0 <= it - 1 < 8:
                    act1(it - 1)
                if 0 <= it - 2 < 8:
                    l2(it - 2)
                    act2(it - 2)
                if 0 <= it - 3 < 8:
                    l3(it - 3)
                    act3(it - 3)
                if 0 <= it - 4 < 8:
                    l4(it - 4)
                if it == 9:
                    l5(0)       # chain a: deps (ACT4a at it7) 2 its old
                if FILL[it]:
                    filler(FILL[it])
            l5(1)
    nc.compile()
    return nc


def _pow2_scale(a, cap=224.0):
    m = float(np.abs(a).max())
    if m == 0.0:
        return 1.0
    return float(2.0 ** np.floor(np.log2(cap / m)))


def _pack(x, w1, b1, w2, b2, w3, b3, w4, b4, w5, b5, drop1, drop2):
    """Build per-sample w1x/wm blobs; return bias/scale data."""
    B = x.shape[0]
    f4 = np.float32
    x3 = np.ascontiguousarray(x.reshape(B, 1024, HW), dtype=f4)
    w1m = w1.reshape(B, 512, 1024).astype(f4, copy=False)
    w2m = w2.reshape(B, 256, 512).astype(f4, copy=False)
    w3m = w3.reshape(B, 128, 256).astype(f4, copy=False)
    w4m = w4.reshape(B, 64, 128).astype(f4, copy=False)
    w5m = w5.reshape(B, 64).astype(f4, copy=False)

    sx = _pow2_scale(x3)
    s1 = _pow2_scale(w1m)
    s2 = _pow2_scale(w2m)
    s3 = _pow2_scale(w3m)

    def chunkT(wT, nk, nm):  # [B, cin, cout] -> [B, 128, nk*nm*128]
        Bn, cin, cout = wT.shape
        return np.ascontiguousarray(
            wT.reshape(Bn, nk, 128, nm, 128).transpose(0, 2, 1, 3, 4)
        ).reshape(Bn, 128, nk * nm * 128)

    w1T = chunkT(np.swapaxes(w1m, 1, 2) * s1, 8, 4)
    xc = np.ascontiguousarray(
        x3.reshape(B, 8, 128, HW).transpose(0, 2, 1, 3)).reshape(B, 128, 512) * sx
    w1x = np.concatenate([xc, w1T], axis=2).astype(F8NP)

    w2T = chunkT(np.swapaxes(w2m, 1, 2) * s2, 4, 2)
    w3T = chunkT(np.swapaxes(w3m, 1, 2) * s3, 2, 1)
    m1 = (drop1.reshape(B, 512, HW) >= np.float32(0.5)).astype(f4) * f4(2.0)
    m1 = np.ascontiguousarray(
        m1.reshape(B, 4, 128, HW).transpose(0, 2, 1, 3)).reshape(B, 128, 256)
    m2 = (drop2.reshape(B, 256, HW) >= np.float32(0.5)).astype(f4) * f4(2.0)
    m2 = np.ascontiguousarray(
        m2.reshape(B, 2, 128, HW).transpose(0, 2, 1, 3)).reshape(B, 128, 128)
    wm = np.concatenate([w2T, w3T, m1, m2], axis=2).astype(F8NP)

    wb = np.zeros((B, 128, WB_PER), f4)
    wb[:, :, 0:64] = np.swapaxes(w4m, 1, 2)
    wb[:, :64, 64] = w5m

    b1s = b1.astype(f4) * f4(s1 * sx)
    b2s = b2.astype(f4) * f4(s2)
    scales = (1.0 / (s1 * sx), 1.0 / s2, 1.0 / s3)
    return w1x, wm, wb, (b1s, b2s, b3.astype(f4), b4.astype(f4),
                         b5.reshape(B).astype(f4)), scales


def kernel(**inputs):
    global _COMPILED, LAST_RESULT
    if _COMPILED is None:
        _COMPILED = _build()
    nc = _COMPILED

    w1x, wm, wb, (b1s, b2s, b3f, b4f, b5f), scales = _pack(
        **{k: np.asarray(v) for k, v in inputs.items()})

    in_maps = []
    for c in range(N_CORES):
        sl = slice(c * S, (c + 1) * S)
        wbc = wb[sl].transpose(1, 0, 2).reshape(128, S * WB_PER)

        cb = np.zeros((8, CB_COLS), np.float32)
        for r in range(8):
            cb[r, OH_OFF + r * HW:OH_OFF + (r + 1) * HW] = 1.0
        for j in range(S):
            g = c * S + j
            cb[0:4, B1_OFF + j * 128:B1_OFF + (j + 1) * 128] = \
                b1s[g].reshape(4, 128)
            cb[0:2, B2_OFF + j * 128:B2_OFF + (j + 1) * 128] = \
                b2s[g].reshape(2, 128)
        cb[0:4, B4_OFF:B4_OFF + 64] = b4f[c * S:c * S + 4]
        cb[0:4, B4_OFF + 64:B4_OFF + 128] = b4f[c * S + 4:c * S + 8]
        cb[0:4, B5_OFF] = b5f[c * S:c * S + 4]
        cb[0:4, B5_OFF + 1] = b5f[c * S + 4:c * S + 8]

        scc = np.empty((128, SC_COLS), np.float32)
        scc[:, 0] = scales[0]
        scc[:, 1] = scales[1]
        scc[:, 2] = scales[2]
        for j in range(S):
            scc[:, 3 + j] = b3f[c * S + j]

        in_maps.append({
            "w1x": np.ascontiguousarray(w1x[sl]),
            "wm": np.ascontiguousarray(wm[sl]),
            "cb": cb.astype(BFNP),
            "sc": scc,
            "wb": np.ascontiguousarray(wbc).astype(BFNP),
        })

    res = None
    for attempt in range(3):
        try:
            res = run_bass_kernel_spmd(nc, in_maps, core_ids=list(range(N_CORES)))
            break
        except Exception:
            if attempt == 2:
                raise
            time.sleep(20)
            try:  # best-effort device reconnect after NRT_EXEC_UNIT_UNRECOVERABLE
                import jax
                jax.clear_caches()
                import jax.extend.backend as _jeb
                _jeb.clear_backends()
            except Exception:
                pass
    LAST_RESULT = res
    outs = [np.asarray(res.results[c]["out"]).reshape(S, 8, 8)
            for c in range(N_CORES)]
    return np.concatenate(outs, axis=0).astype(np.float32)
